# revision 63
# baseline (speedup 1.0000x reference)
"""Trainium2 Bass kernel for nn_MixerModel (Mamba-style mixer).

Sharding: 8 cores = 4 batches x 2-way tensor-parallel split of d_inner.
Each core computes its batch's full residual stream (D=272 feature-major,
features in partitions, time in free dim), the full conv/silu'd xm (so the
xproj contraction is local), and the selective scan for its 272-channel
d_inner shard in a (d,n)-replicated 128-partition layout using the DVE
tensor_tensor_scan instruction.  One 2-way AllReduce per layer after
out_proj.  Only the first 1024 tokens are computed: the reference appends
1024 pad tokens after the real ones and every op is causal, so they cannot
affect the sliced output h[:, :1024].
"""

import math
import sys

sys.setrecursionlimit(200000)

import numpy as np

for _p in ("/opt/trn_rl_repo", "/root/.axon_site/_ro/trn_rl_repo"):
    if _p not in sys.path:
        sys.path.insert(0, _p)

import concourse.bass as bass  # noqa: E402
import concourse.bacc as bacc  # noqa: E402
import concourse.tile as tile  # noqa: E402
import concourse.mybir as mybir  # noqa: E402

F32 = mybir.dt.float32
F32R = mybir.dt.float32r
BF16 = mybir.dt.bfloat16
AF = mybir.ActivationFunctionType
OP = mybir.AluOpType

B, L = 4, 1024
D_MODEL, D_TIME, D = 256, 16, 272
N_LAYER, VOCAB = 4, 1000
D_INNER, D_STATE, D_CONV, DT_RANK = 544, 16, 4, 17
DTP = 18          # DT_RANK padded even (fp32r matmul M must be even)
XPAD = 4          # leading zero cols on xm tiles (causal conv left-pad)
T = 1024          # causal truncation: pad tokens never reach the output
SH = 272          # d_inner shard per core
NTILE = 34        # (SH*16)/128 scan tiles
NC2 = T // 512    # 512-col psum chunks
PERM = np.array([0, 2, 4, 6, 8, 10, 12, 14, 1, 3, 5, 7, 9, 11, 13, 15])
MT = [(0, 128), (128, 128), (256, 16)]            # D=272 row tiles
MT6 = [(0, 128), (128, 128), (256, 16), (272, 128), (400, 128), (528, 16)]
EPS = 1e-5


def _chunks():
    return [(c * 512, 512) for c in range(NC2)]


DEBUG = False


def _mm(nc, out, lhsT, rhs, start=None, stop=None, **kw):
    return nc.tensor.matmul(out, lhsT, rhs, start=start, stop=stop, **kw)


def _probe(tc, name, ap):
    if not DEBUG:
        return
    nc = tc.nc
    shape = [int(s) for s in ap.shape]
    t = nc.dram_tensor(f"dbg_{name}", shape, F32, kind="ExternalOutput").ap()
    nc.sync.dma_start(t[:], ap)


def build_program(n_cores=8):
    nc = bacc.Bacc(
        "TRN2",
        target_bir_lowering=False,
        debug=False,
        enable_asserts=False,
        num_devices=n_cores,
    )
    groups = [[2 * i, 2 * i + 1] for i in range(n_cores // 2)]

    d = {}

    def din(name, shape, dtype=F32):
        d[name] = nc.dram_tensor(name, list(shape), dtype, kind="ExternalInput").ap()

    din("embrows", (256, T))
    din("time_row", (1, T))
    din("divpat", (1, 16))
    din("shiftv", (16, 1))
    din("repsel", (128, 2048), mybir.dt.float32r)
    din("yredsel", (128, 2048), F32R)
    din("c272", (272, 2))
    din("ascT", (4 * 128, NTILE))
    din("win", (4 * 272, 816), F32R)
    din("wxpdt", (4 * 544, DTP), F32R)
    din("wxpB", (4 * 544, 128), F32R)
    din("wxpC", (4 * 544, 128), F32R)
    din("wdt", (4 * DTP, 272), F32R)
    din("wout", (4 * 272, 272), F32R)
    din("convdiag", (4 * 544, 512), F32R)
    din("dtb", (272, 4))
    din("convb", (544, 4))
    din("dparam", (272, 4))
    din("lnw", (4, 272), F32R)
    din("lnbT", (272, 4))
    din("lnfw", (1, 272), F32R)
    din("lnfbT", (272, 1))
    out_fm = nc.dram_tensor("out_fm", [272, T], F32R, kind="ExternalOutput").ap()

    with tile.TileContext(nc) as tc:
        _body(tc, d, out_fm, groups)
    nc.compile()
    return nc


def _body(tc, d, out_fm, groups):
    nc = tc.nc
    from contextlib import ExitStack

    ctx = ExitStack()
    with ctx:
        consts = ctx.enter_context(tc.tile_pool(name="consts", bufs=1))
        wpool = ctx.enter_context(tc.tile_pool(name="wpool", bufs=1))
        act_pool = ctx.enter_context(tc.tile_pool(name="acts", bufs=1))
        scan_pool = ctx.enter_context(tc.tile_pool(name="scan", bufs=2))
        tmp_pool = ctx.enter_context(tc.tile_pool(name="tmps", bufs=2))
        big_pool = ctx.enter_context(tc.tile_pool(name="bigs", bufs=1))
        psA = ctx.enter_context(tc.tile_pool(name="psA", bufs=2, space="PSUM"))
        psB = ctx.enter_context(tc.tile_pool(name="psB", bufs=4, space="PSUM"))
        psY = ctx.enter_context(tc.tile_pool(name="psY", bufs=2, space="PSUM"))
        dram = ctx.enter_context(tc.tile_pool(name="dram", bufs=1, space="DRAM"))

        # ---------- constants ----------
        repsel = consts.tile([128, 2048], F32R)
        yredsel = consts.tile([128, 2048], F32R)
        c272 = []
        for (m0, ml) in MT:
            c_t = consts.tile([ml, 2], F32, name=f"c272_{m0}")
            nc.sync.dma_start(c_t[:], d["c272"][m0:m0 + ml, :])
            c272.append(c_t)

        # ---------- pre-stage: embeddings -> residual r ----------
        r_tiles = []
        for (m0, ml) in MT:
            rt = act_pool.tile([ml, T], F32, name=f"r_{m0}")
            r_tiles.append(rt)

        eps_t = consts.tile([1, 1], F32)
        nc.vector.memset(eps_t[:], EPS)
        zero16 = consts.tile([16, 1], F32)
        nc.vector.memset(zero16[:], 0.0)
        zero4 = consts.tile([128, XPAD], F32)
        nc.vector.memset(zero4[:], 0.0)
        with tc.tile_pool(name="emb", bufs=1) as epool:
            for i in range(2):
                for (c0, cl) in _chunks():
                    nc.sync.dma_start(r_tiles[i][:, c0:c0 + cl],
                                      d["embrows"][128 * i:128 * (i + 1),
                                                   c0:c0 + cl])
                    nc.scalar.activation(r_tiles[i][:, c0:c0 + cl],
                                         r_tiles[i][:, c0:c0 + cl], AF.Tanh)

            # temporal embedding -> r rows 256..271 (8 sin rows, 8 cos rows)
            time_sb = epool.tile([1, T], F32)
            nc.sync.dma_start(time_sb[:], d["time_row"][:])
            divp = epool.tile([1, 16], F32)
            nc.sync.dma_start(divp[:], d["divpat"][:])
            shiftv = epool.tile([16, 1], F32)
            nc.sync.dma_start(shiftv[:], d["shiftv"][:])
            for (c0, cl) in _chunks():
                arg_ps = psA.tile([16, 512], F32, tag="mm", name="arg_ps")
                _mm(nc, arg_ps[:], divp[:], time_sb[:, c0:c0 + cl],
                                 start=True, stop=True)
                argsh = tmp_pool.tile([16, 512], F32, tag="rsq", name="argsh")
                nc.vector.tensor_scalar(argsh[:], arg_ps[:], shiftv[:], None,
                                        op0=OP.add)
                # wrap into [-pi, pi] by 4 halving range wraps (|x| < 16pi)
                wr = tmp_pool.tile([16, 512], F32, tag="t1", name="wr")
                nc.vector.add_range_wrap(wr[:], argsh[:], 0.0,
                                         8 * math.pi, 16 * math.pi)
                for bnd in (4 * math.pi, 2 * math.pi, math.pi):
                    nc.vector.add_range_wrap(wr[:], wr[:], 0.0, bnd, 2 * bnd)
                if c0 == 0:
                    _probe(tc, "argsh", argsh[:])
                    _probe(tc, "wr", wr[:])
                    _probe(tc, "shiftv", shiftv[:])
                nc.scalar.activation(r_tiles[2][0:16, c0:c0 + cl], wr[:],
                                     AF.Sin, bias=zero16[:])

        _probe(tc, "remb0", r_tiles[0][:])
        _probe(tc, "rtemp", r_tiles[2][:])

        # big scan constants: issued after the pre-stage DMAs so they don't
        # delay the embedding/weight loads (only needed ~100us in)
        nc.sync.dma_start(repsel[:], d["repsel"][:])
        nc.sync.dma_start(yredsel[:], d["yredsel"][:])

        # ---------- layers ----------
        for l in range(N_LAYER):
            _layer(tc, d, l, r_tiles, c272, repsel, yredsel,
                   wpool, tmp_pool, big_pool, scan_pool, psA, psB, psY, dram,
                   groups, eps_t, zero4)

        # ---------- final layernorm -> output ----------
        lnfw = consts.tile([1, 272], F32R)
        nc.sync.dma_start(lnfw[:], d["lnfw"][:])
        lnfb = []
        for (m0, ml) in MT:
            t = consts.tile([ml, 1], F32, name=f"lnfb_{m0}")
            nc.sync.dma_start(t[:], d["lnfbT"][m0:m0 + ml, :])
            lnfb.append(t)
        xn_tiles = _layernorm(tc, r_tiles, lnfw, lnfb, c272, tmp_pool, psA,
                              big_pool, eps_t)
        for rt, (m0, ml) in zip(xn_tiles, MT):
            for (c0, cl) in _chunks():
                nc.sync.dma_start(out_fm[m0:m0 + ml, c0:c0 + cl],
                                  rt[:, c0:c0 + cl])


def _layernorm(tc, r_tiles, lnw, lnb_tiles, c272, tmp_pool, psA, big_pool,
               eps_t):
    """Feature-major LN: stats via PE sum-matmuls, apply via outer-products."""
    nc = tc.nc
    rstd_sb = tmp_pool.tile([1, T], F32R, tag="ln_rstd", name="rstd_sb", bufs=1)
    negms_sb = tmp_pool.tile([1, T], F32R, tag="ln_negms", name="negms_sb", bufs=1)
    for (c0, cl) in _chunks():
        nm_ps = psA.tile([1, 512], F32, tag="mm", name="nm_ps")
        sq_ps = psA.tile([1, 512], F32, tag="mm", name="sq_ps")
        for kc, (m0, ml) in enumerate(MT):
            st = (kc == 0)
            sp = (kc == 2)
            _mm(nc, nm_ps[:], c272[kc][:, 0:1],
                             r_tiles[kc][:, c0:c0 + cl], start=st, stop=sp)
            rsq = tmp_pool.tile([ml, 512], F32, tag="rsq", name="rsq")
            nc.scalar.square(rsq[:], r_tiles[kc][:, c0:c0 + cl])
            _mm(nc, sq_ps[:], c272[kc][:, 1:2], rsq[:], start=st, stop=sp)
        m2 = tmp_pool.tile([1, 512], F32, tag="lnst", name="m2")
        nc.scalar.square(m2[:], nm_ps[:])
        # rstd = sqrt(1/(var+eps)); sqrt/square/copy share one act table so
        # the LN region avoids the Ln<->Exp table ping-pong entirely
        var = tmp_pool.tile([1, 512], F32, tag="lnst", name="var")
        nc.vector.scalar_tensor_tensor(var[:], sq_ps[:], eps_t[:], m2[:],
                                       op0=OP.add, op1=OP.subtract)
        rv = tmp_pool.tile([1, 512], F32, tag="lnst", name="rv")
        nc.vector.reciprocal(rv[:], var[:])
        nc.scalar.activation(rstd_sb[:, c0:c0 + cl], rv[:], AF.Sqrt)
        nc.vector.tensor_tensor(negms_sb[:, c0:c0 + cl], nm_ps[:],
                                rstd_sb[:, c0:c0 + cl], OP.mult)
    xn_tiles = []
    for mi, (m0, ml) in enumerate(MT):
        xn = big_pool.tile([ml, T], F32R, tag=f"xn_{m0}", name=f"xn_{m0}")
        for (c0, cl) in _chunks():
            sc_ps = psA.tile([128, 512], F32, tag="mm", name="sc_ps")
            _mm(nc, sc_ps[:ml, :], lnw[:, m0:m0 + ml],
                             rstd_sb[:, c0:c0 + cl], start=True, stop=True)
            t1 = tmp_pool.tile([ml, 512], F32, tag="ln_t1", name="t1")
            nc.vector.tensor_tensor(t1[:], r_tiles[mi][:, c0:c0 + cl],
                                    sc_ps[:ml, :], OP.mult)
            b2_ps = psA.tile([128, 512], F32, tag="mm", name="b2_ps")
            _mm(nc, b2_ps[:ml, :], lnw[:, m0:m0 + ml],
                             negms_sb[:, c0:c0 + cl], start=True, stop=True)
            nc.vector.scalar_tensor_tensor(
                xn[:, c0:c0 + cl], t1[:], lnb_tiles[mi][:], b2_ps[:ml, :],
                op0=OP.add, op1=OP.add)
        xn_tiles.append(xn)
    return xn_tiles


def _layer(tc, d, l, r_tiles, c272, repsel, yredsel,
           wpool, tmp_pool, big_pool, scan_pool, psA, psB, psY, dram, groups,
           eps_t, zero4):
    nc = tc.nc

    # -------- per-layer weights to SBUF --------
    lnw = wpool.tile([1, 272], F32R, tag="lnw", name="lnw")
    nc.sync.dma_start(lnw[:], d["lnw"][l:l + 1, :])
    lnb = []
    for (m0, ml) in MT:
        t = wpool.tile([ml, 1], F32, tag=f"lnb{m0}", name=f"lnb{m0}")
        nc.sync.dma_start(t[:], d["lnbT"][m0:m0 + ml, l:l + 1])
        lnb.append(t)
    win_sb = []
    for kc, (m0, ml) in enumerate(MT):
        t = wpool.tile([ml, 816], F32R, tag=f"win{kc}", name=f"win{kc}")
        nc.sync.dma_start(t[:], d["win"][272 * l + m0:272 * l + m0 + ml, :])
        win_sb.append(t)
    wxp = {}
    for nm, w in (("dt", DTP), ("B", 128), ("C", 128)):
        lst = []
        for kc, (k0, kl) in enumerate(MT6):
            t = wpool.tile([kl, w], F32R, tag=f"wxp{nm}{kc}", name=f"wxp{nm}{kc}")
            nc.sync.dma_start(t[:], d[f"wxp{nm}"][544 * l + k0:544 * l + k0 + kl, :])
            lst.append(t)
        wxp[nm] = lst
    wdt_sb = wpool.tile([DTP, 272], F32R, tag="wdt", name="wdt_sb")
    nc.sync.dma_start(wdt_sb[:], d["wdt"][DTP * l:DTP * (l + 1), :])
    wout_sb = []
    for kc, (k0, kl) in enumerate(MT):
        t = wpool.tile([kl, 272], F32R, tag=f"wout{kc}", name=f"wout{kc}")
        nc.sync.dma_start(t[:], d["wout"][272 * l + k0:272 * l + k0 + kl, :])
        wout_sb.append(t)
    vecs = {}
    for nm, dn in (("dtb", "dtb"), ("dp", "dparam")):
        lst = []
        for (m0, ml) in MT:
            t = wpool.tile([ml, 1], F32, tag=f"{nm}{m0}", name=f"{nm}{m0}")
            nc.sync.dma_start(t[:], d[dn][m0:m0 + ml, l:l + 1])
            lst.append(t)
        vecs[nm] = lst
    convb = []
    for (m0, ml) in MT6:
        t = wpool.tile([ml, 1], F32, tag=f"convb{m0}", name=f"convb{m0}")
        nc.sync.dma_start(t[:], d["convb"][m0:m0 + ml, l:l + 1])
        convb.append(t)
    asc = wpool.tile([128, NTILE], F32, tag="asc", name="asc")
    nc.sync.dma_start(asc[:], d["ascT"][128 * l:128 * (l + 1), :])

    # -------- LN --------
    xn_tiles = _layernorm(tc, r_tiles, lnw, lnb, c272, tmp_pool, psA, big_pool,
                          eps_t)
    if l == 0:
        _probe(tc, "xn0", xn_tiles[0][:])

    # -------- in_proj: xn -> xm (6 tiles, shard-first order) + silu(z) --------
    xm_tiles = []
    sz_tiles = []
    for mi in range(9):
        m0 = 272 * (mi // 3) + MT[mi % 3][0]
        ml = MT[mi % 3][1]
        dest_xm = mi < 6
        if dest_xm:
            ot = big_pool.tile([ml, T + XPAD], F32R, tag=f"xm{mi}",
                               name=f"xm{mi}")
            xm_tiles.append(ot)
            nc.scalar.copy(ot[:, 0:XPAD], zero4[:ml, :])
        else:
            ot = big_pool.tile([ml, T], BF16, tag=f"sz{mi}", name=f"sz{mi}")
            sz_tiles.append(ot)
        for (c0, cl) in _chunks():
            ps = psA.tile([128, 512], F32, tag="mm", name="ip_ps")
            for kc in range(3):
                _mm(nc, ps[:ml, :], win_sb[kc][:, m0:m0 + ml],
                                 xn_tiles[kc][:, c0:c0 + cl],
                                 start=(kc == 0), stop=(kc == 2))
            if dest_xm:
                nc.scalar.copy(ot[:, XPAD + c0:XPAD + c0 + cl], ps[:ml, :])
            else:
                sg = tmp_pool.tile([ml, 512], F32, tag="sg", name="sg")
                nc.scalar.activation(sg[:], ps[:ml, :], AF.Sigmoid)
                nc.vector.tensor_tensor(ot[:, c0:c0 + cl], ps[:ml, :], sg[:],
                                        OP.mult)

    # -------- depthwise causal conv (PE diag matmuls) + silu -> u --------
    u_tiles = []
    for mi, (m0, ml) in enumerate(MT6):
        ut = big_pool.tile([ml, T], F32R, tag=f"u{mi}", name=f"u{mi}")
        u_tiles.append(ut)
        cdg = tmp_pool.tile([ml, 4 * ml], F32R, tag="cdg", name=f"cdg{mi}")
        nc.sync.dma_start(cdg[:],
                          d["convdiag"][544 * l + m0:544 * l + m0 + ml,
                                        0:4 * ml])
        for (c0, cl) in _chunks():
            acc_ps = psY.tile([128, 512], F32, tag="y", name="cv_ps")
            # xm is left-padded with XPAD zero cols: xm_pad[:, XPAD+j] = xm[j],
            # so tap t reads xm_pad[:, c0+1+t : c0+1+t+cl] (causal conv)
            for tap in range(4):
                _mm(nc, acc_ps[:ml, :], cdg[:, tap * ml:tap * ml + ml],
                    xm_tiles[mi][:, c0 + 1 + tap:c0 + 1 + tap + cl],
                    start=(tap == 0), stop=(tap == 3))
            sg = tmp_pool.tile([ml, 512], F32, tag="sg", name="sg")
            nc.scalar.activation(sg[:], acc_ps[:ml, :], AF.Sigmoid,
                                 bias=convb[mi][:])
            nc.vector.scalar_tensor_tensor(ut[:, c0:c0 + cl],
                                           acc_ps[:ml, :],
                                           convb[mi][:], sg[:],
                                           op0=OP.add, op1=OP.mult)

    if l == 0:
        _probe(tc, "xm0", xm_tiles[0][:])
        _probe(tc, "u0", u_tiles[0][:])
        _probe(tc, "sz0", sz_tiles[0][:])

    # -------- xproj: u -> dt rows, B_rep, C_rep --------
    dt_sb = tmp_pool.tile([DTP, T], F32R, tag="cvacc", name="dt_sb", bufs=1)
    brep = big_pool.tile([128, T], BF16, tag="brep", name="brep")
    crep = big_pool.tile([128, T], BF16, tag="crep", name="crep")
    for nm, ot, w in (("dt", dt_sb, DTP), ("B", brep, 128), ("C", crep, 128)):
        for (c0, cl) in _chunks():
            ps = psA.tile([128, 512], F32, tag="mm", name="xp_ps")
            for kc in range(6):
                kl = MT6[kc][1]
                _mm(nc, ps[:w, :], wxp[nm][kc][:, 0:w],
                                 u_tiles[kc][:, c0:c0 + cl],
                                 start=(kc == 0), stop=(kc == 5))
            nc.scalar.copy(ot[:, c0:c0 + cl], ps[:w, :])

    # -------- dt_proj + softplus -> delta; du = delta * u_own --------
    delta_tiles = []
    deltar_tiles = []
    du_tiles = []
    # softplus(x) = ln(1 + exp(x)): batch all EXPs then all LNs; the act
    # table chooser picks first-fit sets, so alternation would reload tables
    for mi, (m0, ml) in enumerate(MT):
        dl_t = big_pool.tile([ml, T + XPAD], F32R, tag=f"xm{mi}",
                             name=f"delta{mi}")
        delta_tiles.append(dl_t)
        deltar_tiles.append(dl_t)
        for (c0, cl) in _chunks():
            ps = psA.tile([128, 512], F32, tag="mm", name="dt_ps")
            _mm(nc, ps[:ml, :], wdt_sb[:, m0:m0 + ml],
                             dt_sb[:, c0:c0 + cl], start=True, stop=True)
            nc.scalar.activation(dl_t[:, c0:c0 + cl], ps[:ml, :], AF.Exp,
                                 bias=vecs["dtb"][mi][:])
    for mi, (m0, ml) in enumerate(MT):
        dl_t = delta_tiles[mi]
        for (c0, cl) in _chunks():
            nc.scalar.activation(dl_t[:, c0:c0 + cl], dl_t[:, c0:c0 + cl],
                                 AF.Ln, bias=1.0)
        du_t = big_pool.tile([ml, T + XPAD], F32R, tag=f"xm{mi + 3}",
                             name=f"du{mi}")
        nc.gpsimd.tensor_tensor(du_t[:, 0:T], dl_t[:, 0:T],
                                u_tiles[mi][:, 0:T], OP.mult)
        du_tiles.append(du_t)

    if l == 0:
        _probe(tc, "dtsb", dt_sb[:])
        _probe(tc, "brep", brep[:])
        _probe(tc, "crep", crep[:])
        _probe(tc, "delta0", delta_tiles[0][:])
        _probe(tc, "du0", du_tiles[0][:])

    # -------- selective scan over 34 (d,n)-tiles --------
    # group 2 (1 pair) first so out_proj's first psum accumulation input is
    # ready long before groups 0/1 finish
    ysz_tiles = {}
    for g in (2, 0, 1):
        gm0, gml = MT[g]
        y_ps_c = [psY.tile([128, 512], F32, tag="y", name=f"y_ps{c}")
                  for c in range(NC2)]
        k_lo, k_hi = 16 * g, min(16 * g + 16, NTILE)
        # process scan tiles in PAIRS: one [128, 2T] scan + one Pool hC per
        # pair (the a-column at the pair seam is zeroed so the recurrence
        # restarts), halving per-tile instruction and semaphore counts
        for kp in range(k_lo, k_hi, 2):
            rt = kp // 16
            kl = MT[rt][1]
            a_t = scan_pool.tile([128, 2 * T], F32, tag="a", name="a_t",
                                 bufs=2)
            b_t = scan_pool.tile([128, 2 * T], F32, tag="b", name="b_t",
                                 bufs=1)
            h_t = scan_pool.tile([128, 2 * T], F32R, tag="h", name="h_t",
                                 bufs=2)
            for ki in range(2):
                k = kp + ki
                j = k - k_lo
                o0 = ki * T
                dr_ps = []
                for (c0, cl) in _chunks():
                    ps1 = psB.tile([128, 512], F32, tag="rep", name="dr_ps")
                    _mm(nc, ps1[:], repsel[:kl, 128 * j:128 * j + 128],
                                     deltar_tiles[rt][:, c0:c0 + cl],
                                     start=True, stop=True)
                    dr_ps.append(ps1)
                for ci, (c0, cl) in enumerate(_chunks()):
                    nc.scalar.activation(a_t[:, o0 + c0:o0 + c0 + cl],
                                         dr_ps[ci][:], AF.Exp,
                                         scale=asc[:, k:k + 1])
                    ps2 = psB.tile([128, 512], F32, tag="rep", name="du_ps")
                    _mm(nc, ps2[:], repsel[:kl, 128 * j:128 * j + 128],
                                     du_tiles[rt][:, c0:c0 + cl],
                                     start=True, stop=True)
                    nc.vector.tensor_tensor(b_t[:, o0 + c0:o0 + c0 + cl],
                                            ps2[:], brep[:, c0:c0 + cl],
                                            OP.mult)
            nc.vector.memset(a_t[:, T:T + 1], 0.0)
            nc.vector.tensor_tensor_scan(h_t[:], a_t[:], b_t[:], 0.0,
                                         op0=OP.mult, op1=OP.add)
            nc.gpsimd.tensor_tensor(h_t[:, 0:T], h_t[:, 0:T], crep[:],
                                    OP.mult)
            nc.gpsimd.tensor_tensor(h_t[:, T:2 * T], h_t[:, T:2 * T],
                                    crep[:], OP.mult)
            for ki in range(2):
                k = kp + ki
                j = k - k_lo
                o0 = ki * T
                for ci, (c0, cl) in enumerate(_chunks()):
                    _mm(nc, y_ps_c[ci][:gml, :],
                                     yredsel[:, 128 * j:128 * j + gml],
                                     h_t[:, o0 + c0:o0 + c0 + cl],
                                     start=(j == 0), stop=(k == k_hi - 1))
        t2 = big_pool.tile([gml, T], F32R, tag=f"u{3 + g}", name=f"yt{g}")
        for ci, (c0, cl) in enumerate(_chunks()):
            nc.vector.scalar_tensor_tensor(
                t2[:, c0:c0 + cl], u_tiles[g][:, c0:c0 + cl],
                vecs["dp"][g][:], y_ps_c[ci][:gml, :],
                op0=OP.mult, op1=OP.add)
        nc.gpsimd.tensor_tensor(t2[:], t2[:], sz_tiles[g][:], OP.mult)
        if l == 0 and g == 0:
            _probe(tc, "ysz0", t2[:])
        ysz_tiles[g] = t2

    # -------- out_proj -> chunked AllReduce -> residual add --------
    # Per 512-col chunk: out_proj, AllReduce, residual add.  Chunk c0's
    # collective overlaps chunk c1's out_proj, and the next layer's LN /
    # in_proj on c0 can start while c1's collective is in flight.
    for ci, (c0, cl) in enumerate(_chunks()):
        ar_in = dram.tile([272, 512], BF16, tag=f"ar_in{l}_{ci}",
                          name=f"ar_in{l}_{ci}")
        ar_out = dram.tile([272, 512], BF16, tag=f"ar_out{l}_{ci}",
                           name=f"ar_out{l}_{ci}")
        for mi, (m0, ml) in enumerate(MT):
            ps = psA.tile([128, 512], F32, tag="mm", name="op_ps")
            for i, kc in enumerate((2, 0, 1)):
                _mm(nc, ps[:ml, :], wout_sb[kc][:, m0:m0 + ml],
                                 ysz_tiles[kc][:, c0:c0 + cl],
                                 start=(i == 0), stop=(i == 2))
            op_sb = tmp_pool.tile([ml, 512], BF16, tag="opc", name=f"op{mi}")
            nc.scalar.copy(op_sb[:], ps[:ml, :])
            nc.sync.dma_start(ar_in[m0:m0 + ml, :], op_sb[:])
        nc.gpsimd.collective_compute(
            "AllReduce", OP.add, replica_groups=groups,
            ins=[ar_in.opt()], outs=[ar_out.opt()])
        for mi, (m0, ml) in enumerate(MT):
            os_t = tmp_pool.tile([ml, 512], BF16, tag="osc", name=f"os{mi}")
            nc.sync.dma_start(os_t[:], ar_out[m0:m0 + ml, :])
            nc.gpsimd.tensor_tensor(r_tiles[mi][:, c0:c0 + cl],
                                    r_tiles[mi][:, c0:c0 + cl], os_t[:],
                                    OP.add)
    if l == 0:
        _probe(tc, "r0after", r_tiles[0][:])


# ======================= host side =======================

def prep_core_inputs(inputs, core):
    f32 = np.float32
    beta, s = core // 2, core % 2
    g = lambda k: np.asarray(inputs[k], f32)
    type_seq = np.asarray(inputs["type_seq"]).astype(np.int64)
    rows = np.r_[np.arange(272 * s, 272 * s + 272),
                 np.arange(272 * (1 - s), 272 * (1 - s) + 272)]  # own-first xm order
    sh = np.arange(272 * s, 272 * s + 272)

    m = {}
    # embedding gather is pure indexing; tanh stays on device
    m["embrows"] = np.ascontiguousarray(g("emb")[type_seq[beta]].T)
    m["time_row"] = np.ascontiguousarray(g("time_seq")[beta][None, :T])
    div = np.exp(np.arange(0, D_TIME, 2, dtype=f32) * (-(math.log(10000.0) / D_TIME)))
    m["divpat"] = np.tile(div, 2)[None, :].astype(f32)
    m["shiftv"] = np.r_[np.full(8, 0.0), np.full(8, 0.5 * math.pi)][:, None].astype(f32)

    repsel = np.zeros((128, 2048), f32)
    yredsel = np.zeros((128, 2048), f32)
    for j in range(16):
        for p in range(128):
            repsel[8 * j + p // 16, 128 * j + p] = 1.0
            yredsel[p, 128 * j + 8 * j + p // 16] = 1.0
    m["repsel"] = repsel
    m["yredsel"] = yredsel
    c272 = np.empty((272, 2), f32)
    c272[:, 0] = -1.0 / D
    c272[:, 1] = 1.0 / D
    m["c272"] = c272

    A = -np.exp(g("A_log"))  # (4, 544, 16)
    ascT = np.empty((4 * 128, NTILE), f32)
    for l in range(4):
        for k in range(NTILE):
            for p in range(128):
                ascT[128 * l + p, k] = A[l, sh[8 * k + p // 16], p % 16]
    m["ascT"] = ascT

    in_w = g("in_w").copy()       # (4, 1088, 272)
    in_w[:, :, 256:272] = in_w[:, :, 256 + PERM]
    win = np.empty((4 * 272, 816), f32)
    for l in range(4):
        W = in_w[l][np.r_[rows, 544 + sh]]  # (816, 272)
        win[272 * l:272 * (l + 1)] = W.T
    m["win"] = win

    xp = g("xproj_w")             # (4, 49, 544)
    DTP = 18
    wxpdt = np.zeros((4 * 544, DTP), f32)
    wxpB = np.zeros((4 * 544, 128), f32)
    wxpC = np.zeros((4 * 544, 128), f32)
    for l in range(4):
        Wx = xp[l][:, rows]       # (49, 544) reordered columns
        wxpdt[544 * l:544 * (l + 1), :DT_RANK] = Wx[:DT_RANK].T
        for p in range(128):
            wxpB[544 * l:544 * (l + 1), p] = Wx[DT_RANK + p % 16]
            wxpC[544 * l:544 * (l + 1), p] = Wx[DT_RANK + 16 + p % 16]
    m["wxpdt"], m["wxpB"], m["wxpC"] = wxpdt, wxpB, wxpC

    dt_w = g("dt_w")              # (4, 544, 17)
    wdt = np.zeros((4 * DTP, 272), f32)
    for l in range(4):
        wdt[DTP * l:DTP * l + DT_RANK] = dt_w[l][sh].T
    m["wdt"] = wdt

    out_w = g("out_w").copy()     # (4, 272, 544)
    out_w[:, 256:272, :] = out_w[:, 256 + PERM, :]
    wout = np.empty((4 * 272, 272), f32)
    for l in range(4):
        wout[272 * l:272 * (l + 1)] = out_w[l][:, sh].T
    m["wout"] = wout

    cw = g("conv_w")[:, :, 0, :]  # (4, 544, 4)
    convdiag = np.zeros((4 * 544, 512), f32)
    mt6 = [(0, 128), (128, 128), (256, 16), (272, 128), (400, 128), (528, 16)]
    for l in range(4):
        base = cw[l][rows]  # (544, 4) own-first channel order
        for (m0, ml) in mt6:
            for tap in range(4):
                blk = convdiag[544 * l + m0:544 * l + m0 + ml,
                               tap * ml:(tap + 1) * ml]
                np.fill_diagonal(blk, base[m0:m0 + ml, tap])
    m["convdiag"] = convdiag

    m["dtb"] = np.ascontiguousarray(g("dt_b")[:, sh].T)          # (272, 4)
    m["convb"] = np.ascontiguousarray(g("conv_b")[:, rows].T)    # (544, 4)
    m["dparam"] = np.ascontiguousarray(g("D_param")[:, sh].T)    # (272, 4)

    lnw = g("norm_w").copy()
    lnb = g("norm_b").copy()
    lnw[:, 256:272] = lnw[:, 256 + PERM]
    lnb[:, 256:272] = lnb[:, 256 + PERM]
    m["lnw"] = lnw
    m["lnbT"] = np.ascontiguousarray(lnb.T)
    lnfw = g("normf_w").copy()
    lnfb = g("normf_b").copy()
    lnfw[256:272] = lnfw[256 + PERM]
    lnfb[256:272] = lnfb[256 + PERM]
    m["lnfw"] = lnfw[None, :]
    m["lnfbT"] = lnfb[:, None]
    return m


def assemble_output(per_core_results):
    """per_core_results: list of 8 dicts with 'out_fm' (272, 1024)."""
    outs = []
    for b in range(B):
        o = np.asarray(per_core_results[2 * b]["out_fm"], np.float32)
        oo = o.copy()
        oo[256 + PERM] = o[256:272]
        outs.append(oo.T)
    return np.stack(outs).astype(np.float32)


_PROG = {}


def _get_program(n_cores=8):
    if n_cores not in _PROG:
        _PROG[n_cores] = build_program(n_cores)
    return _PROG[n_cores]


def run(inputs, trace=False, trace_kwargs=None):
    nc = _get_program(8)
    in_maps = [prep_core_inputs(inputs, c) for c in range(8)]
    from concourse.bass_utils import run_bass_kernel_spmd
    res = run_bass_kernel_spmd(nc, in_maps, core_ids=list(range(8)),
                               trace=trace, **(trace_kwargs or {}))
    return assemble_output(res.results), res


def kernel(**inputs):
    out, _ = run(inputs)
    return out



# revision 65
# speedup vs baseline: 1.0264x; 1.0264x over previous
"""Trainium2 Bass kernel for nn_MixerModel (Mamba-style mixer).

Sharding: 8 cores = 4 batches x 2-way tensor-parallel split of d_inner.
Each core computes its batch's full residual stream (D=272 feature-major,
features in partitions, time in free dim), the full conv/silu'd xm (so the
xproj contraction is local), and the selective scan for its 272-channel
d_inner shard in a (d,n)-replicated 128-partition layout using the DVE
tensor_tensor_scan instruction.  One 2-way AllReduce per layer after
out_proj.  Only the first 1024 tokens are computed: the reference appends
1024 pad tokens after the real ones and every op is causal, so they cannot
affect the sliced output h[:, :1024].
"""

import math
import sys

sys.setrecursionlimit(200000)

import numpy as np

for _p in ("/opt/trn_rl_repo", "/root/.axon_site/_ro/trn_rl_repo"):
    if _p not in sys.path:
        sys.path.insert(0, _p)

import concourse.bass as bass  # noqa: E402
import concourse.bacc as bacc  # noqa: E402
import concourse.tile as tile  # noqa: E402
import concourse.mybir as mybir  # noqa: E402

F32 = mybir.dt.float32
F32R = mybir.dt.float32r
BF16 = mybir.dt.bfloat16
AF = mybir.ActivationFunctionType
OP = mybir.AluOpType

B, L = 4, 1024
D_MODEL, D_TIME, D = 256, 16, 272
N_LAYER, VOCAB = 4, 1000
D_INNER, D_STATE, D_CONV, DT_RANK = 544, 16, 4, 17
DTP = 18          # DT_RANK padded even (fp32r matmul M must be even)
XPAD = 4          # leading zero cols on xm tiles (causal conv left-pad)
T = 1024          # causal truncation: pad tokens never reach the output
SH = 272          # d_inner shard per core
NTILE = 34        # (SH*16)/128 scan tiles
NC2 = T // 512    # 512-col psum chunks
PERM = np.array([0, 2, 4, 6, 8, 10, 12, 14, 1, 3, 5, 7, 9, 11, 13, 15])
MT = [(0, 128), (128, 128), (256, 16)]            # D=272 row tiles
MT6 = [(0, 128), (128, 128), (256, 16), (272, 128), (400, 128), (528, 16)]
EPS = 1e-5


def _chunks():
    return [(c * 512, 512) for c in range(NC2)]


DEBUG = False


def _mm(nc, out, lhsT, rhs, start=None, stop=None, **kw):
    return nc.tensor.matmul(out, lhsT, rhs, start=start, stop=stop, **kw)


def _probe(tc, name, ap):
    if not DEBUG:
        return
    nc = tc.nc
    shape = [int(s) for s in ap.shape]
    t = nc.dram_tensor(f"dbg_{name}", shape, F32, kind="ExternalOutput").ap()
    nc.sync.dma_start(t[:], ap)


def build_program(n_cores=8):
    nc = bacc.Bacc(
        "TRN2",
        target_bir_lowering=False,
        debug=False,
        enable_asserts=False,
        num_devices=n_cores,
    )
    groups = [[2 * i, 2 * i + 1] for i in range(n_cores // 2)]

    d = {}

    def din(name, shape, dtype=F32):
        d[name] = nc.dram_tensor(name, list(shape), dtype, kind="ExternalInput").ap()

    din("embrows", (256, T))
    din("time_row", (1, T))
    din("divpat", (1, 16))
    din("shiftv", (16, 1))
    din("repsel", (128, 2048), mybir.dt.float32r)
    din("yredsel", (128, 2048), F32R)
    din("c272", (272, 2))
    din("ascT", (4 * 128, NTILE))
    din("win", (4 * 272, 816), F32R)
    din("wxpdt", (4 * 544, DTP), F32R)
    din("wxpB", (4 * 544, 128), F32R)
    din("wxpC", (4 * 544, 128), F32R)
    din("wdt", (4 * DTP, 272), F32R)
    din("wout", (4 * 272, 272), F32R)
    din("convdiag", (4 * 544, 512), F32R)
    din("dtb", (272, 4))
    din("convb", (544, 4))
    din("dparam", (272, 4))
    din("lnw", (4, 272), F32R)
    din("lnbT", (272, 4))
    din("lnfw", (1, 272), F32R)
    din("lnfbT", (272, 1))
    out_fm = nc.dram_tensor("out_fm", [272, T], F32R, kind="ExternalOutput").ap()

    with tile.TileContext(nc) as tc:
        _body(tc, d, out_fm, groups)
    nc.compile()
    return nc


def _body(tc, d, out_fm, groups):
    nc = tc.nc
    from contextlib import ExitStack

    ctx = ExitStack()
    with ctx:
        consts = ctx.enter_context(tc.tile_pool(name="consts", bufs=1))
        wpool = ctx.enter_context(tc.tile_pool(name="wpool", bufs=1))
        act_pool = ctx.enter_context(tc.tile_pool(name="acts", bufs=1))
        scan_pool = ctx.enter_context(tc.tile_pool(name="scan", bufs=2))
        tmp_pool = ctx.enter_context(tc.tile_pool(name="tmps", bufs=2))
        big_pool = ctx.enter_context(tc.tile_pool(name="bigs", bufs=1))
        psA = ctx.enter_context(tc.tile_pool(name="psA", bufs=2, space="PSUM"))
        psB = ctx.enter_context(tc.tile_pool(name="psB", bufs=4, space="PSUM"))
        psY = ctx.enter_context(tc.tile_pool(name="psY", bufs=2, space="PSUM"))
        dram = ctx.enter_context(tc.tile_pool(name="dram", bufs=1, space="DRAM"))

        # ---------- constants ----------
        repsel = consts.tile([128, 2048], F32R)
        yredsel = consts.tile([128, 2048], F32R)
        c272 = []
        for (m0, ml) in MT:
            c_t = consts.tile([ml, 2], F32, name=f"c272_{m0}")
            nc.sync.dma_start(c_t[:], d["c272"][m0:m0 + ml, :])
            c272.append(c_t)

        # ---------- pre-stage: embeddings -> residual r ----------
        r_tiles = []
        for (m0, ml) in MT:
            rt = act_pool.tile([ml, T], F32, name=f"r_{m0}")
            r_tiles.append(rt)

        eps_t = consts.tile([1, 1], F32)
        nc.vector.memset(eps_t[:], EPS)
        zero16 = consts.tile([16, 1], F32)
        nc.vector.memset(zero16[:], 0.0)
        zero4 = consts.tile([128, XPAD], F32)
        nc.vector.memset(zero4[:], 0.0)
        with tc.tile_pool(name="emb", bufs=1) as epool:
            for i in range(2):
                for (c0, cl) in _chunks():
                    nc.sync.dma_start(r_tiles[i][:, c0:c0 + cl],
                                      d["embrows"][128 * i:128 * (i + 1),
                                                   c0:c0 + cl])
                    nc.scalar.activation(r_tiles[i][:, c0:c0 + cl],
                                         r_tiles[i][:, c0:c0 + cl], AF.Tanh)

            # temporal embedding -> r rows 256..271 (8 sin rows, 8 cos rows)
            time_sb = epool.tile([1, T], F32)
            nc.sync.dma_start(time_sb[:], d["time_row"][:])
            divp = epool.tile([1, 16], F32)
            nc.sync.dma_start(divp[:], d["divpat"][:])
            shiftv = epool.tile([16, 1], F32)
            nc.sync.dma_start(shiftv[:], d["shiftv"][:])
            for (c0, cl) in _chunks():
                arg_ps = psA.tile([16, 512], F32, tag="mm", name="arg_ps")
                _mm(nc, arg_ps[:], divp[:], time_sb[:, c0:c0 + cl],
                                 start=True, stop=True)
                argsh = tmp_pool.tile([16, 512], F32, tag="rsq", name="argsh")
                nc.vector.tensor_scalar(argsh[:], arg_ps[:], shiftv[:], None,
                                        op0=OP.add)
                # wrap into [-pi, pi] by 4 halving range wraps (|x| < 16pi)
                wr = tmp_pool.tile([16, 512], F32, tag="t1", name="wr")
                nc.vector.add_range_wrap(wr[:], argsh[:], 0.0,
                                         8 * math.pi, 16 * math.pi)
                for bnd in (4 * math.pi, 2 * math.pi, math.pi):
                    nc.vector.add_range_wrap(wr[:], wr[:], 0.0, bnd, 2 * bnd)
                if c0 == 0:
                    _probe(tc, "argsh", argsh[:])
                    _probe(tc, "wr", wr[:])
                    _probe(tc, "shiftv", shiftv[:])
                nc.scalar.activation(r_tiles[2][0:16, c0:c0 + cl], wr[:],
                                     AF.Sin, bias=zero16[:])

        _probe(tc, "remb0", r_tiles[0][:])
        _probe(tc, "rtemp", r_tiles[2][:])

        # big scan constants: issued after the pre-stage DMAs so they don't
        # delay the embedding/weight loads (only needed ~100us in)
        nc.sync.dma_start(repsel[:], d["repsel"][:])
        nc.sync.dma_start(yredsel[:], d["yredsel"][:])

        # ---------- layers ----------
        for l in range(N_LAYER):
            _layer(tc, d, l, r_tiles, c272, repsel, yredsel,
                   wpool, tmp_pool, big_pool, scan_pool, psA, psB, psY, dram,
                   groups, eps_t, zero4)

        # ---------- final layernorm -> output ----------
        lnfw = consts.tile([1, 272], F32R)
        nc.sync.dma_start(lnfw[:], d["lnfw"][:])
        lnfb = []
        for (m0, ml) in MT:
            t = consts.tile([ml, 1], F32, name=f"lnfb_{m0}")
            nc.sync.dma_start(t[:], d["lnfbT"][m0:m0 + ml, :])
            lnfb.append(t)
        xn_tiles = _layernorm(tc, r_tiles, lnfw, lnfb, c272, tmp_pool, psA,
                              big_pool, eps_t)
        for rt, (m0, ml) in zip(xn_tiles, MT):
            for (c0, cl) in _chunks():
                nc.sync.dma_start(out_fm[m0:m0 + ml, c0:c0 + cl],
                                  rt[:, c0:c0 + cl])


def _layernorm(tc, r_tiles, lnw, lnb_tiles, c272, tmp_pool, psA, big_pool,
               eps_t):
    """Feature-major LN: stats via PE sum-matmuls, apply via outer-products."""
    nc = tc.nc
    rstd_sb = tmp_pool.tile([1, T], F32R, tag="ln_rstd", name="rstd_sb", bufs=1)
    negms_sb = tmp_pool.tile([1, T], F32R, tag="ln_negms", name="negms_sb", bufs=1)
    for (c0, cl) in _chunks():
        nm_ps = psA.tile([1, 512], F32, tag="mm", name="nm_ps")
        sq_ps = psA.tile([1, 512], F32, tag="mm", name="sq_ps")
        for kc, (m0, ml) in enumerate(MT):
            st = (kc == 0)
            sp = (kc == 2)
            _mm(nc, nm_ps[:], c272[kc][:, 0:1],
                             r_tiles[kc][:, c0:c0 + cl], start=st, stop=sp)
            rsq = tmp_pool.tile([ml, 512], F32, tag="rsq", name="rsq")
            nc.scalar.square(rsq[:], r_tiles[kc][:, c0:c0 + cl])
            _mm(nc, sq_ps[:], c272[kc][:, 1:2], rsq[:], start=st, stop=sp)
        m2 = tmp_pool.tile([1, 512], F32, tag="lnst", name="m2")
        nc.scalar.square(m2[:], nm_ps[:])
        # rstd = sqrt(1/(var+eps)); sqrt/square/copy share one act table so
        # the LN region avoids the Ln<->Exp table ping-pong entirely
        var = tmp_pool.tile([1, 512], F32, tag="lnst", name="var")
        nc.vector.scalar_tensor_tensor(var[:], sq_ps[:], eps_t[:], m2[:],
                                       op0=OP.add, op1=OP.subtract)
        rv = tmp_pool.tile([1, 512], F32, tag="lnst", name="rv")
        nc.vector.reciprocal(rv[:], var[:])
        nc.scalar.activation(rstd_sb[:, c0:c0 + cl], rv[:], AF.Sqrt)
        nc.vector.tensor_tensor(negms_sb[:, c0:c0 + cl], nm_ps[:],
                                rstd_sb[:, c0:c0 + cl], OP.mult)
    xn_tiles = []
    for mi, (m0, ml) in enumerate(MT):
        xn = big_pool.tile([ml, T], F32R, tag=f"xn_{m0}", name=f"xn_{m0}")
        for (c0, cl) in _chunks():
            sc_ps = psA.tile([128, 512], F32, tag="mm", name="sc_ps")
            _mm(nc, sc_ps[:ml, :], lnw[:, m0:m0 + ml],
                             rstd_sb[:, c0:c0 + cl], start=True, stop=True)
            t1 = tmp_pool.tile([ml, 512], F32, tag="ln_t1", name="t1")
            nc.vector.tensor_tensor(t1[:], r_tiles[mi][:, c0:c0 + cl],
                                    sc_ps[:ml, :], OP.mult)
            b2_ps = psA.tile([128, 512], F32, tag="mm", name="b2_ps")
            _mm(nc, b2_ps[:ml, :], lnw[:, m0:m0 + ml],
                             negms_sb[:, c0:c0 + cl], start=True, stop=True)
            nc.vector.scalar_tensor_tensor(
                xn[:, c0:c0 + cl], t1[:], lnb_tiles[mi][:], b2_ps[:ml, :],
                op0=OP.add, op1=OP.add)
        xn_tiles.append(xn)
    return xn_tiles


def _layer(tc, d, l, r_tiles, c272, repsel, yredsel,
           wpool, tmp_pool, big_pool, scan_pool, psA, psB, psY, dram, groups,
           eps_t, zero4):
    nc = tc.nc

    # -------- per-layer weights to SBUF --------
    lnw = wpool.tile([1, 272], F32R, tag="lnw", name="lnw")
    nc.sync.dma_start(lnw[:], d["lnw"][l:l + 1, :])
    lnb = []
    for (m0, ml) in MT:
        t = wpool.tile([ml, 1], F32, tag=f"lnb{m0}", name=f"lnb{m0}")
        nc.sync.dma_start(t[:], d["lnbT"][m0:m0 + ml, l:l + 1])
        lnb.append(t)
    win_sb = []
    for kc, (m0, ml) in enumerate(MT):
        t = wpool.tile([ml, 816], F32R, tag=f"win{kc}", name=f"win{kc}")
        nc.sync.dma_start(t[:], d["win"][272 * l + m0:272 * l + m0 + ml, :])
        win_sb.append(t)
    wxp = {}
    for nm, w in (("dt", DTP), ("B", 128), ("C", 128)):
        lst = []
        for kc, (k0, kl) in enumerate(MT6):
            t = wpool.tile([kl, w], F32R, tag=f"wxp{nm}{kc}", name=f"wxp{nm}{kc}")
            nc.sync.dma_start(t[:], d[f"wxp{nm}"][544 * l + k0:544 * l + k0 + kl, :])
            lst.append(t)
        wxp[nm] = lst
    wdt_sb = wpool.tile([DTP, 272], F32R, tag="wdt", name="wdt_sb")
    nc.sync.dma_start(wdt_sb[:], d["wdt"][DTP * l:DTP * (l + 1), :])
    wout_sb = []
    for kc, (k0, kl) in enumerate(MT):
        t = wpool.tile([kl, 272], F32R, tag=f"wout{kc}", name=f"wout{kc}")
        nc.sync.dma_start(t[:], d["wout"][272 * l + k0:272 * l + k0 + kl, :])
        wout_sb.append(t)
    vecs = {}
    for nm, dn in (("dtb", "dtb"), ("dp", "dparam")):
        lst = []
        for (m0, ml) in MT:
            t = wpool.tile([ml, 1], F32, tag=f"{nm}{m0}", name=f"{nm}{m0}")
            nc.sync.dma_start(t[:], d[dn][m0:m0 + ml, l:l + 1])
            lst.append(t)
        vecs[nm] = lst
    convb = []
    for (m0, ml) in MT6:
        t = wpool.tile([ml, 1], F32, tag=f"convb{m0}", name=f"convb{m0}")
        nc.sync.dma_start(t[:], d["convb"][m0:m0 + ml, l:l + 1])
        convb.append(t)
    asc = wpool.tile([128, NTILE], F32, tag="asc", name="asc")
    nc.sync.dma_start(asc[:], d["ascT"][128 * l:128 * (l + 1), :])

    # -------- LN --------
    xn_tiles = _layernorm(tc, r_tiles, lnw, lnb, c272, tmp_pool, psA, big_pool,
                          eps_t)
    if l == 0:
        _probe(tc, "xn0", xn_tiles[0][:])

    # -------- in_proj: xn -> xm (6 tiles, shard-first order) + silu(z) --------
    xm_tiles = []
    sz_tiles = []
    for mi in range(9):
        m0 = 272 * (mi // 3) + MT[mi % 3][0]
        ml = MT[mi % 3][1]
        dest_xm = mi < 6
        if dest_xm:
            ot = big_pool.tile([ml, T + XPAD], F32R, tag=f"xm{mi}",
                               name=f"xm{mi}")
            xm_tiles.append(ot)
            nc.scalar.copy(ot[:, 0:XPAD], zero4[:ml, :])
        else:
            ot = big_pool.tile([ml, T], BF16, tag=f"sz{mi}", name=f"sz{mi}")
            sz_tiles.append(ot)
        for (c0, cl) in _chunks():
            ps = psA.tile([128, 512], F32, tag="mm", name="ip_ps")
            for kc in range(3):
                _mm(nc, ps[:ml, :], win_sb[kc][:, m0:m0 + ml],
                                 xn_tiles[kc][:, c0:c0 + cl],
                                 start=(kc == 0), stop=(kc == 2))
            if dest_xm:
                nc.scalar.copy(ot[:, XPAD + c0:XPAD + c0 + cl], ps[:ml, :])
            else:
                sg = tmp_pool.tile([ml, 512], F32, tag="sg", name="sg")
                nc.scalar.activation(sg[:], ps[:ml, :], AF.Sigmoid)
                nc.vector.tensor_tensor(ot[:, c0:c0 + cl], ps[:ml, :], sg[:],
                                        OP.mult)

    # -------- depthwise causal conv (PE diag matmuls) + silu -> u --------
    u_tiles = []
    for mi, (m0, ml) in enumerate(MT6):
        ut = big_pool.tile([ml, T], F32R, tag=f"u{mi}", name=f"u{mi}")
        u_tiles.append(ut)
        cdg = tmp_pool.tile([ml, 4 * ml], F32R, tag="cdg", name=f"cdg{mi}")
        nc.sync.dma_start(cdg[:],
                          d["convdiag"][544 * l + m0:544 * l + m0 + ml,
                                        0:4 * ml])
        for (c0, cl) in _chunks():
            acc_ps = psY.tile([128, 512], F32, tag="y", name="cv_ps")
            # xm is left-padded with XPAD zero cols: xm_pad[:, XPAD+j] = xm[j],
            # so tap t reads xm_pad[:, c0+1+t : c0+1+t+cl] (causal conv)
            for tap in range(4):
                _mm(nc, acc_ps[:ml, :], cdg[:, tap * ml:tap * ml + ml],
                    xm_tiles[mi][:, c0 + 1 + tap:c0 + 1 + tap + cl],
                    start=(tap == 0), stop=(tap == 3))
            sg = tmp_pool.tile([ml, 512], F32, tag="sg", name="sg")
            nc.scalar.activation(sg[:], acc_ps[:ml, :], AF.Sigmoid,
                                 bias=convb[mi][:])
            nc.vector.scalar_tensor_tensor(ut[:, c0:c0 + cl],
                                           acc_ps[:ml, :],
                                           convb[mi][:], sg[:],
                                           op0=OP.add, op1=OP.mult)

    if l == 0:
        _probe(tc, "xm0", xm_tiles[0][:])
        _probe(tc, "u0", u_tiles[0][:])
        _probe(tc, "sz0", sz_tiles[0][:])

    # -------- xproj: u -> dt rows, B_rep, C_rep --------
    dt_sb = tmp_pool.tile([DTP, T], F32R, tag="cvacc", name="dt_sb", bufs=1)
    brep = big_pool.tile([128, T], BF16, tag="brep", name="brep")
    crep = big_pool.tile([128, T], BF16, tag="crep", name="crep")
    for nm, ot, w in (("dt", dt_sb, DTP), ("B", brep, 128), ("C", crep, 128)):
        for (c0, cl) in _chunks():
            ps = psA.tile([128, 512], F32, tag="mm", name="xp_ps")
            for kc in range(6):
                kl = MT6[kc][1]
                _mm(nc, ps[:w, :], wxp[nm][kc][:, 0:w],
                                 u_tiles[kc][:, c0:c0 + cl],
                                 start=(kc == 0), stop=(kc == 5))
            nc.scalar.copy(ot[:, c0:c0 + cl], ps[:w, :])

    # -------- dt_proj + softplus -> delta; du = delta * u_own --------
    delta_tiles = []
    deltar_tiles = []
    du_tiles = []
    # softplus(x) = ln(1 + exp(x)): batch all EXPs then all LNs; the act
    # table chooser picks first-fit sets, so alternation would reload tables
    for mi, (m0, ml) in enumerate(MT):
        dl_t = big_pool.tile([ml, T + XPAD], F32R, tag=f"xm{mi}",
                             name=f"delta{mi}")
        delta_tiles.append(dl_t)
        deltar_tiles.append(dl_t)
        for (c0, cl) in _chunks():
            ps = psA.tile([128, 512], F32, tag="mm", name="dt_ps")
            _mm(nc, ps[:ml, :], wdt_sb[:, m0:m0 + ml],
                             dt_sb[:, c0:c0 + cl], start=True, stop=True)
            nc.scalar.activation(dl_t[:, c0:c0 + cl], ps[:ml, :], AF.Exp,
                                 bias=vecs["dtb"][mi][:])
    for mi, (m0, ml) in enumerate(MT):
        dl_t = delta_tiles[mi]
        for (c0, cl) in _chunks():
            nc.scalar.activation(dl_t[:, c0:c0 + cl], dl_t[:, c0:c0 + cl],
                                 AF.Ln, bias=1.0)
        du_t = big_pool.tile([ml, T + XPAD], F32R, tag=f"xm{mi + 3}",
                             name=f"du{mi}")
        nc.gpsimd.tensor_tensor(du_t[:, 0:T], dl_t[:, 0:T],
                                u_tiles[mi][:, 0:T], OP.mult)
        du_tiles.append(du_t)

    if l == 0:
        _probe(tc, "dtsb", dt_sb[:])
        _probe(tc, "brep", brep[:])
        _probe(tc, "crep", crep[:])
        _probe(tc, "delta0", delta_tiles[0][:])
        _probe(tc, "du0", du_tiles[0][:])

    # -------- selective scan over 34 (d,n)-tiles --------
    ysz_tiles = {}
    for g in range(3):
        gm0, gml = MT[g]
        y_ps_c = [psY.tile([128, 512], F32, tag="y", name=f"y_ps{c}")
                  for c in range(NC2)]
        k_lo, k_hi = 16 * g, min(16 * g + 16, NTILE)
        # process scan tiles in PAIRS: one [128, 2T] scan + one Pool hC per
        # pair (the a-column at the pair seam is zeroed so the recurrence
        # restarts), halving per-tile instruction and semaphore counts
        for kp in range(k_lo, k_hi, 2):
            rt = kp // 16
            kl = MT[rt][1]
            a_t = scan_pool.tile([128, 2 * T], F32, tag="a", name="a_t",
                                 bufs=2)
            b_t = scan_pool.tile([128, 2 * T], F32, tag="b", name="b_t",
                                 bufs=1)
            h_t = scan_pool.tile([128, 2 * T], F32R, tag="h", name="h_t",
                                 bufs=2)
            for ki in range(2):
                k = kp + ki
                j = k - k_lo
                o0 = ki * T
                dr_ps = []
                for (c0, cl) in _chunks():
                    ps1 = psB.tile([128, 512], F32, tag="rep", name="dr_ps")
                    _mm(nc, ps1[:], repsel[:kl, 128 * j:128 * j + 128],
                                     deltar_tiles[rt][:, c0:c0 + cl],
                                     start=True, stop=True)
                    dr_ps.append(ps1)
                for ci, (c0, cl) in enumerate(_chunks()):
                    nc.scalar.activation(a_t[:, o0 + c0:o0 + c0 + cl],
                                         dr_ps[ci][:], AF.Exp,
                                         scale=asc[:, k:k + 1])
                    ps2 = psB.tile([128, 512], F32, tag="rep", name="du_ps")
                    _mm(nc, ps2[:], repsel[:kl, 128 * j:128 * j + 128],
                                     du_tiles[rt][:, c0:c0 + cl],
                                     start=True, stop=True)
                    nc.vector.tensor_tensor(b_t[:, o0 + c0:o0 + c0 + cl],
                                            ps2[:], brep[:, c0:c0 + cl],
                                            OP.mult)
            nc.vector.memset(a_t[:, T:T + 1], 0.0)
            nc.vector.tensor_tensor_scan(h_t[:], a_t[:], b_t[:], 0.0,
                                         op0=OP.mult, op1=OP.add)
            nc.gpsimd.tensor_tensor(h_t[:, 0:T], h_t[:, 0:T], crep[:],
                                    OP.mult)
            nc.gpsimd.tensor_tensor(h_t[:, T:2 * T], h_t[:, T:2 * T],
                                    crep[:], OP.mult)
            for ki in range(2):
                k = kp + ki
                j = k - k_lo
                o0 = ki * T
                for ci, (c0, cl) in enumerate(_chunks()):
                    _mm(nc, y_ps_c[ci][:gml, :],
                                     yredsel[:, 128 * j:128 * j + gml],
                                     h_t[:, o0 + c0:o0 + c0 + cl],
                                     start=(j == 0), stop=(k == k_hi - 1))
        t2 = big_pool.tile([gml, T], F32R, tag=f"u{3 + g}", name=f"yt{g}")
        for ci, (c0, cl) in enumerate(_chunks()):
            nc.vector.scalar_tensor_tensor(
                t2[:, c0:c0 + cl], u_tiles[g][:, c0:c0 + cl],
                vecs["dp"][g][:], y_ps_c[ci][:gml, :],
                op0=OP.mult, op1=OP.add)
        nc.gpsimd.tensor_tensor(t2[:], t2[:], sz_tiles[g][:], OP.mult)
        if l == 0 and g == 0:
            _probe(tc, "ysz0", t2[:])
        ysz_tiles[g] = t2

    # -------- out_proj -> chunked AllReduce -> residual add --------
    # Per 512-col chunk: out_proj, AllReduce, residual add.  Chunk c0's
    # collective overlaps chunk c1's out_proj, and the next layer's LN /
    # in_proj on c0 can start while c1's collective is in flight.
    for ci, (c0, cl) in enumerate(_chunks()):
        ar_in = dram.tile([272, 512], BF16, tag=f"ar_in{l}_{ci}",
                          name=f"ar_in{l}_{ci}")
        ar_out = dram.tile([272, 512], BF16, tag=f"ar_out{l}_{ci}",
                           name=f"ar_out{l}_{ci}")
        for mi, (m0, ml) in enumerate(MT):
            ps = psA.tile([128, 512], F32, tag="mm", name="op_ps")
            for i, kc in enumerate(range(3)):
                _mm(nc, ps[:ml, :], wout_sb[kc][:, m0:m0 + ml],
                                 ysz_tiles[kc][:, c0:c0 + cl],
                                 start=(i == 0), stop=(i == 2))
            op_sb = tmp_pool.tile([ml, 512], BF16, tag="opc", name=f"op{mi}")
            nc.scalar.copy(op_sb[:], ps[:ml, :])
            nc.sync.dma_start(ar_in[m0:m0 + ml, :], op_sb[:])
        nc.gpsimd.collective_compute(
            "AllReduce", OP.add, replica_groups=groups,
            ins=[ar_in.opt()], outs=[ar_out.opt()])
        for mi, (m0, ml) in enumerate(MT):
            os_t = tmp_pool.tile([ml, 512], BF16, tag="osc", name=f"os{mi}")
            nc.sync.dma_start(os_t[:], ar_out[m0:m0 + ml, :])
            # on DVE, not Pool: the next chunk's collective trigger shares
            # Pool's FIFO and must not queue behind these adds
            nc.vector.tensor_tensor(r_tiles[mi][:, c0:c0 + cl],
                                    r_tiles[mi][:, c0:c0 + cl], os_t[:],
                                    OP.add)
    if l == 0:
        _probe(tc, "r0after", r_tiles[0][:])


# ======================= host side =======================

def prep_core_inputs(inputs, core):
    f32 = np.float32
    beta, s = core // 2, core % 2
    g = lambda k: np.asarray(inputs[k], f32)
    type_seq = np.asarray(inputs["type_seq"]).astype(np.int64)
    rows = np.r_[np.arange(272 * s, 272 * s + 272),
                 np.arange(272 * (1 - s), 272 * (1 - s) + 272)]  # own-first xm order
    sh = np.arange(272 * s, 272 * s + 272)

    m = {}
    # embedding gather is pure indexing; tanh stays on device
    m["embrows"] = np.ascontiguousarray(g("emb")[type_seq[beta]].T)
    m["time_row"] = np.ascontiguousarray(g("time_seq")[beta][None, :T])
    div = np.exp(np.arange(0, D_TIME, 2, dtype=f32) * (-(math.log(10000.0) / D_TIME)))
    m["divpat"] = np.tile(div, 2)[None, :].astype(f32)
    m["shiftv"] = np.r_[np.full(8, 0.0), np.full(8, 0.5 * math.pi)][:, None].astype(f32)

    repsel = np.zeros((128, 2048), f32)
    yredsel = np.zeros((128, 2048), f32)
    for j in range(16):
        for p in range(128):
            repsel[8 * j + p // 16, 128 * j + p] = 1.0
            yredsel[p, 128 * j + 8 * j + p // 16] = 1.0
    m["repsel"] = repsel
    m["yredsel"] = yredsel
    c272 = np.empty((272, 2), f32)
    c272[:, 0] = -1.0 / D
    c272[:, 1] = 1.0 / D
    m["c272"] = c272

    A = -np.exp(g("A_log"))  # (4, 544, 16)
    ascT = np.empty((4 * 128, NTILE), f32)
    for l in range(4):
        for k in range(NTILE):
            for p in range(128):
                ascT[128 * l + p, k] = A[l, sh[8 * k + p // 16], p % 16]
    m["ascT"] = ascT

    in_w = g("in_w").copy()       # (4, 1088, 272)
    in_w[:, :, 256:272] = in_w[:, :, 256 + PERM]
    win = np.empty((4 * 272, 816), f32)
    for l in range(4):
        W = in_w[l][np.r_[rows, 544 + sh]]  # (816, 272)
        win[272 * l:272 * (l + 1)] = W.T
    m["win"] = win

    xp = g("xproj_w")             # (4, 49, 544)
    DTP = 18
    wxpdt = np.zeros((4 * 544, DTP), f32)
    wxpB = np.zeros((4 * 544, 128), f32)
    wxpC = np.zeros((4 * 544, 128), f32)
    for l in range(4):
        Wx = xp[l][:, rows]       # (49, 544) reordered columns
        wxpdt[544 * l:544 * (l + 1), :DT_RANK] = Wx[:DT_RANK].T
        for p in range(128):
            wxpB[544 * l:544 * (l + 1), p] = Wx[DT_RANK + p % 16]
            wxpC[544 * l:544 * (l + 1), p] = Wx[DT_RANK + 16 + p % 16]
    m["wxpdt"], m["wxpB"], m["wxpC"] = wxpdt, wxpB, wxpC

    dt_w = g("dt_w")              # (4, 544, 17)
    wdt = np.zeros((4 * DTP, 272), f32)
    for l in range(4):
        wdt[DTP * l:DTP * l + DT_RANK] = dt_w[l][sh].T
    m["wdt"] = wdt

    out_w = g("out_w").copy()     # (4, 272, 544)
    out_w[:, 256:272, :] = out_w[:, 256 + PERM, :]
    wout = np.empty((4 * 272, 272), f32)
    for l in range(4):
        wout[272 * l:272 * (l + 1)] = out_w[l][:, sh].T
    m["wout"] = wout

    cw = g("conv_w")[:, :, 0, :]  # (4, 544, 4)
    convdiag = np.zeros((4 * 544, 512), f32)
    mt6 = [(0, 128), (128, 128), (256, 16), (272, 128), (400, 128), (528, 16)]
    for l in range(4):
        base = cw[l][rows]  # (544, 4) own-first channel order
        for (m0, ml) in mt6:
            for tap in range(4):
                blk = convdiag[544 * l + m0:544 * l + m0 + ml,
                               tap * ml:(tap + 1) * ml]
                np.fill_diagonal(blk, base[m0:m0 + ml, tap])
    m["convdiag"] = convdiag

    m["dtb"] = np.ascontiguousarray(g("dt_b")[:, sh].T)          # (272, 4)
    m["convb"] = np.ascontiguousarray(g("conv_b")[:, rows].T)    # (544, 4)
    m["dparam"] = np.ascontiguousarray(g("D_param")[:, sh].T)    # (272, 4)

    lnw = g("norm_w").copy()
    lnb = g("norm_b").copy()
    lnw[:, 256:272] = lnw[:, 256 + PERM]
    lnb[:, 256:272] = lnb[:, 256 + PERM]
    m["lnw"] = lnw
    m["lnbT"] = np.ascontiguousarray(lnb.T)
    lnfw = g("normf_w").copy()
    lnfb = g("normf_b").copy()
    lnfw[256:272] = lnfw[256 + PERM]
    lnfb[256:272] = lnfb[256 + PERM]
    m["lnfw"] = lnfw[None, :]
    m["lnfbT"] = lnfb[:, None]
    return m


def assemble_output(per_core_results):
    """per_core_results: list of 8 dicts with 'out_fm' (272, 1024)."""
    outs = []
    for b in range(B):
        o = np.asarray(per_core_results[2 * b]["out_fm"], np.float32)
        oo = o.copy()
        oo[256 + PERM] = o[256:272]
        outs.append(oo.T)
    return np.stack(outs).astype(np.float32)


_PROG = {}


def _get_program(n_cores=8):
    if n_cores not in _PROG:
        _PROG[n_cores] = build_program(n_cores)
    return _PROG[n_cores]


def run(inputs, trace=False, trace_kwargs=None):
    nc = _get_program(8)
    in_maps = [prep_core_inputs(inputs, c) for c in range(8)]
    from concourse.bass_utils import run_bass_kernel_spmd
    res = run_bass_kernel_spmd(nc, in_maps, core_ids=list(range(8)),
                               trace=trace, **(trace_kwargs or {}))
    return assemble_output(res.results), res


def kernel(**inputs):
    out, _ = run(inputs)
    return out



# revision 66
# speedup vs baseline: 1.0353x; 1.0087x over previous
"""Trainium2 Bass kernel for nn_MixerModel (Mamba-style mixer).

Sharding: 8 cores = 4 batches x 2-way tensor-parallel split of d_inner.
Each core computes its batch's full residual stream (D=272 feature-major,
features in partitions, time in free dim), the full conv/silu'd xm (so the
xproj contraction is local), and the selective scan for its 272-channel
d_inner shard in a (d,n)-replicated 128-partition layout using the DVE
tensor_tensor_scan instruction.  One 2-way AllReduce per layer after
out_proj.  Only the first 1024 tokens are computed: the reference appends
1024 pad tokens after the real ones and every op is causal, so they cannot
affect the sliced output h[:, :1024].
"""

import math
import sys

sys.setrecursionlimit(200000)

import numpy as np

for _p in ("/opt/trn_rl_repo", "/root/.axon_site/_ro/trn_rl_repo"):
    if _p not in sys.path:
        sys.path.insert(0, _p)

import concourse.bass as bass  # noqa: E402
import concourse.bacc as bacc  # noqa: E402
import concourse.tile as tile  # noqa: E402
import concourse.mybir as mybir  # noqa: E402

F32 = mybir.dt.float32
F32R = mybir.dt.float32r
BF16 = mybir.dt.bfloat16
AF = mybir.ActivationFunctionType
OP = mybir.AluOpType

B, L = 4, 1024
D_MODEL, D_TIME, D = 256, 16, 272
N_LAYER, VOCAB = 4, 1000
D_INNER, D_STATE, D_CONV, DT_RANK = 544, 16, 4, 17
DTP = 18          # DT_RANK padded even (fp32r matmul M must be even)
XPAD = 4          # leading zero cols on xm tiles (causal conv left-pad)
T = 1024          # causal truncation: pad tokens never reach the output
SH = 272          # d_inner shard per core
NTILE = 34        # (SH*16)/128 scan tiles
NC2 = T // 512    # 512-col psum chunks
PERM = np.array([0, 2, 4, 6, 8, 10, 12, 14, 1, 3, 5, 7, 9, 11, 13, 15])
MT = [(0, 128), (128, 128), (256, 16)]            # D=272 row tiles
MT6 = [(0, 128), (128, 128), (256, 16), (272, 128), (400, 128), (528, 16)]
EPS = 1e-5


def _chunks():
    return [(c * 512, 512) for c in range(NC2)]


DEBUG = False


def _mm(nc, out, lhsT, rhs, start=None, stop=None, **kw):
    return nc.tensor.matmul(out, lhsT, rhs, start=start, stop=stop, **kw)


def _probe(tc, name, ap):
    if not DEBUG:
        return
    nc = tc.nc
    shape = [int(s) for s in ap.shape]
    t = nc.dram_tensor(f"dbg_{name}", shape, F32, kind="ExternalOutput").ap()
    nc.sync.dma_start(t[:], ap)


def build_program(n_cores=8):
    nc = bacc.Bacc(
        "TRN2",
        target_bir_lowering=False,
        debug=False,
        enable_asserts=False,
        num_devices=n_cores,
    )
    groups = [[2 * i, 2 * i + 1] for i in range(n_cores // 2)]

    d = {}

    def din(name, shape, dtype=F32):
        d[name] = nc.dram_tensor(name, list(shape), dtype, kind="ExternalInput").ap()

    din("embrows", (256, T))
    din("time_row", (1, T))
    din("divpat", (1, 16))
    din("shiftv", (16, 1))
    din("repsel", (128, 2048), mybir.dt.float32r)
    din("yredsel", (128, 2048), F32R)
    din("c272", (272, 2))
    din("ascT", (4 * 128, NTILE))
    din("win", (4 * 272, 816), F32R)
    din("wxpdt", (4 * 544, DTP), F32R)
    din("wxpB", (4 * 544, 128), F32R)
    din("wxpC", (4 * 544, 128), F32R)
    din("wdt", (4 * DTP, 272), F32R)
    din("wout", (4 * 272, 272), F32R)
    din("convdiag", (4 * 544, 512), F32R)
    din("dtb", (272, 4))
    din("convb", (544, 4))
    din("dparam", (272, 4))
    din("lnw", (4, 272), F32R)
    din("lnbT", (272, 4))
    din("lnfw", (1, 272), F32R)
    din("lnfbT", (272, 1))
    out_fm = nc.dram_tensor("out_fm", [272, T], F32R, kind="ExternalOutput").ap()

    with tile.TileContext(nc) as tc:
        _body(tc, d, out_fm, groups)
    nc.compile()
    return nc


def _body(tc, d, out_fm, groups):
    nc = tc.nc
    from contextlib import ExitStack

    ctx = ExitStack()
    with ctx:
        consts = ctx.enter_context(tc.tile_pool(name="consts", bufs=1))
        wpool = ctx.enter_context(tc.tile_pool(name="wpool", bufs=1))
        act_pool = ctx.enter_context(tc.tile_pool(name="acts", bufs=1))
        scan_pool = ctx.enter_context(tc.tile_pool(name="scan", bufs=2))
        tmp_pool = ctx.enter_context(tc.tile_pool(name="tmps", bufs=2))
        big_pool = ctx.enter_context(tc.tile_pool(name="bigs", bufs=1))
        psA = ctx.enter_context(tc.tile_pool(name="psA", bufs=2, space="PSUM"))
        psB = ctx.enter_context(tc.tile_pool(name="psB", bufs=4, space="PSUM"))
        psY = ctx.enter_context(tc.tile_pool(name="psY", bufs=2, space="PSUM"))
        dram = ctx.enter_context(tc.tile_pool(name="dram", bufs=1, space="DRAM"))

        # ---------- constants ----------
        repsel = consts.tile([128, 2048], F32R)
        yredsel = consts.tile([128, 2048], F32R)
        c272 = []
        for (m0, ml) in MT:
            c_t = consts.tile([ml, 2], F32, name=f"c272_{m0}")
            nc.sync.dma_start(c_t[:], d["c272"][m0:m0 + ml, :])
            c272.append(c_t)

        # ---------- pre-stage: embeddings -> residual r ----------
        r_tiles = []
        for (m0, ml) in MT:
            rt = act_pool.tile([ml, T], F32, name=f"r_{m0}")
            r_tiles.append(rt)

        eps_t = consts.tile([1, 1], F32)
        nc.vector.memset(eps_t[:], EPS)
        zero16 = consts.tile([16, 1], F32)
        nc.vector.memset(zero16[:], 0.0)
        zero4 = consts.tile([128, XPAD], F32)
        nc.vector.memset(zero4[:], 0.0)
        with tc.tile_pool(name="emb", bufs=1) as epool:
            for i in range(2):
                for (c0, cl) in _chunks():
                    nc.sync.dma_start(r_tiles[i][:, c0:c0 + cl],
                                      d["embrows"][128 * i:128 * (i + 1),
                                                   c0:c0 + cl])
                    nc.scalar.activation(r_tiles[i][:, c0:c0 + cl],
                                         r_tiles[i][:, c0:c0 + cl], AF.Tanh)

            # temporal embedding -> r rows 256..271 (8 sin rows, 8 cos rows)
            time_sb = epool.tile([1, T], F32)
            nc.sync.dma_start(time_sb[:], d["time_row"][:])
            divp = epool.tile([1, 16], F32)
            nc.sync.dma_start(divp[:], d["divpat"][:])
            shiftv = epool.tile([16, 1], F32)
            nc.sync.dma_start(shiftv[:], d["shiftv"][:])
            for (c0, cl) in _chunks():
                arg_ps = psA.tile([16, 512], F32, tag="mm", name="arg_ps")
                _mm(nc, arg_ps[:], divp[:], time_sb[:, c0:c0 + cl],
                                 start=True, stop=True)
                argsh = tmp_pool.tile([16, 512], F32, tag="rsq", name="argsh")
                nc.vector.tensor_scalar(argsh[:], arg_ps[:], shiftv[:], None,
                                        op0=OP.add)
                # wrap into [-pi, pi] by 4 halving range wraps (|x| < 16pi)
                wr = tmp_pool.tile([16, 512], F32, tag="t1", name="wr")
                nc.vector.add_range_wrap(wr[:], argsh[:], 0.0,
                                         8 * math.pi, 16 * math.pi)
                for bnd in (4 * math.pi, 2 * math.pi, math.pi):
                    nc.vector.add_range_wrap(wr[:], wr[:], 0.0, bnd, 2 * bnd)
                if c0 == 0:
                    _probe(tc, "argsh", argsh[:])
                    _probe(tc, "wr", wr[:])
                    _probe(tc, "shiftv", shiftv[:])
                nc.scalar.activation(r_tiles[2][0:16, c0:c0 + cl], wr[:],
                                     AF.Sin, bias=zero16[:])

        _probe(tc, "remb0", r_tiles[0][:])
        _probe(tc, "rtemp", r_tiles[2][:])

        # big scan constants: issued after the pre-stage DMAs so they don't
        # delay the embedding/weight loads (only needed ~100us in)
        nc.sync.dma_start(repsel[:], d["repsel"][:])
        nc.sync.dma_start(yredsel[:], d["yredsel"][:])

        # ---------- layers ----------
        for l in range(N_LAYER):
            _layer(tc, d, l, r_tiles, c272, repsel, yredsel,
                   wpool, tmp_pool, big_pool, scan_pool, psA, psB, psY, dram,
                   groups, eps_t, zero4)

        # ---------- final layernorm -> output ----------
        lnfw = consts.tile([1, 272], F32R)
        nc.sync.dma_start(lnfw[:], d["lnfw"][:])
        lnfb = []
        for (m0, ml) in MT:
            t = consts.tile([ml, 1], F32, name=f"lnfb_{m0}")
            nc.sync.dma_start(t[:], d["lnfbT"][m0:m0 + ml, :])
            lnfb.append(t)
        xn_tiles = _layernorm(tc, r_tiles, lnfw, lnfb, c272, tmp_pool, psA,
                              big_pool, eps_t)
        for rt, (m0, ml) in zip(xn_tiles, MT):
            for (c0, cl) in _chunks():
                nc.sync.dma_start(out_fm[m0:m0 + ml, c0:c0 + cl],
                                  rt[:, c0:c0 + cl])


def _layernorm(tc, r_tiles, lnw, lnb_tiles, c272, tmp_pool, psA, big_pool,
               eps_t):
    """Feature-major LN: stats via PE sum-matmuls, apply via outer-products."""
    nc = tc.nc
    rstd_sb = tmp_pool.tile([1, T], F32R, tag="ln_rstd", name="rstd_sb", bufs=1)
    negms_sb = tmp_pool.tile([1, T], F32R, tag="ln_negms", name="negms_sb", bufs=1)
    for (c0, cl) in _chunks():
        nm_ps = psA.tile([1, 512], F32, tag="mm", name="nm_ps")
        sq_ps = psA.tile([1, 512], F32, tag="mm", name="sq_ps")
        for kc, (m0, ml) in enumerate(MT):
            st = (kc == 0)
            sp = (kc == 2)
            _mm(nc, nm_ps[:], c272[kc][:, 0:1],
                             r_tiles[kc][:, c0:c0 + cl], start=st, stop=sp)
            rsq = tmp_pool.tile([ml, 512], F32, tag="rsq", name="rsq")
            nc.scalar.square(rsq[:], r_tiles[kc][:, c0:c0 + cl])
            _mm(nc, sq_ps[:], c272[kc][:, 1:2], rsq[:], start=st, stop=sp)
        m2 = tmp_pool.tile([1, 512], F32, tag="lnst", name="m2")
        nc.scalar.square(m2[:], nm_ps[:])
        # rstd = sqrt(1/(var+eps)); sqrt/square/copy share one act table so
        # the LN region avoids the Ln<->Exp table ping-pong entirely
        var = tmp_pool.tile([1, 512], F32, tag="lnst", name="var")
        nc.vector.scalar_tensor_tensor(var[:], sq_ps[:], eps_t[:], m2[:],
                                       op0=OP.add, op1=OP.subtract)
        rv = tmp_pool.tile([1, 512], F32, tag="lnst", name="rv")
        nc.vector.reciprocal(rv[:], var[:])
        nc.scalar.activation(rstd_sb[:, c0:c0 + cl], rv[:], AF.Sqrt)
        nc.vector.tensor_tensor(negms_sb[:, c0:c0 + cl], nm_ps[:],
                                rstd_sb[:, c0:c0 + cl], OP.mult)
    xn_tiles = []
    for mi, (m0, ml) in enumerate(MT):
        xn = big_pool.tile([ml, T], F32R, tag=f"xn_{m0}", name=f"xn_{m0}")
        for (c0, cl) in _chunks():
            sc_ps = psA.tile([128, 512], F32, tag="mm", name="sc_ps")
            _mm(nc, sc_ps[:ml, :], lnw[:, m0:m0 + ml],
                             rstd_sb[:, c0:c0 + cl], start=True, stop=True)
            t1 = tmp_pool.tile([ml, 512], F32, tag="ln_t1", name="t1")
            nc.vector.tensor_tensor(t1[:], r_tiles[mi][:, c0:c0 + cl],
                                    sc_ps[:ml, :], OP.mult)
            b2_ps = psA.tile([128, 512], F32, tag="mm", name="b2_ps")
            _mm(nc, b2_ps[:ml, :], lnw[:, m0:m0 + ml],
                             negms_sb[:, c0:c0 + cl], start=True, stop=True)
            nc.vector.scalar_tensor_tensor(
                xn[:, c0:c0 + cl], t1[:], lnb_tiles[mi][:], b2_ps[:ml, :],
                op0=OP.add, op1=OP.add)
        xn_tiles.append(xn)
    return xn_tiles


def _layer(tc, d, l, r_tiles, c272, repsel, yredsel,
           wpool, tmp_pool, big_pool, scan_pool, psA, psB, psY, dram, groups,
           eps_t, zero4):
    nc = tc.nc

    # -------- per-layer weights to SBUF --------
    lnw = wpool.tile([1, 272], F32R, tag="lnw", name="lnw")
    nc.sync.dma_start(lnw[:], d["lnw"][l:l + 1, :])
    lnb = []
    for (m0, ml) in MT:
        t = wpool.tile([ml, 1], F32, tag=f"lnb{m0}", name=f"lnb{m0}")
        nc.sync.dma_start(t[:], d["lnbT"][m0:m0 + ml, l:l + 1])
        lnb.append(t)
    win_sb = []
    for kc, (m0, ml) in enumerate(MT):
        t = wpool.tile([ml, 816], F32R, tag=f"win{kc}", name=f"win{kc}")
        nc.sync.dma_start(t[:], d["win"][272 * l + m0:272 * l + m0 + ml, :])
        win_sb.append(t)
    wxp = {}
    for nm, w in (("dt", DTP), ("B", 128), ("C", 128)):
        lst = []
        for kc, (k0, kl) in enumerate(MT6):
            t = wpool.tile([kl, w], F32R, tag=f"wxp{nm}{kc}", name=f"wxp{nm}{kc}")
            nc.sync.dma_start(t[:], d[f"wxp{nm}"][544 * l + k0:544 * l + k0 + kl, :])
            lst.append(t)
        wxp[nm] = lst
    wdt_sb = wpool.tile([DTP, 272], F32R, tag="wdt", name="wdt_sb")
    nc.sync.dma_start(wdt_sb[:], d["wdt"][DTP * l:DTP * (l + 1), :])
    wout_sb = []
    for kc, (k0, kl) in enumerate(MT):
        t = wpool.tile([kl, 272], F32R, tag=f"wout{kc}", name=f"wout{kc}")
        nc.sync.dma_start(t[:], d["wout"][272 * l + k0:272 * l + k0 + kl, :])
        wout_sb.append(t)
    vecs = {}
    for nm, dn in (("dtb", "dtb"), ("dp", "dparam")):
        lst = []
        for (m0, ml) in MT:
            t = wpool.tile([ml, 1], F32, tag=f"{nm}{m0}", name=f"{nm}{m0}")
            nc.sync.dma_start(t[:], d[dn][m0:m0 + ml, l:l + 1])
            lst.append(t)
        vecs[nm] = lst
    convb = []
    for (m0, ml) in MT6:
        t = wpool.tile([ml, 1], F32, tag=f"convb{m0}", name=f"convb{m0}")
        nc.sync.dma_start(t[:], d["convb"][m0:m0 + ml, l:l + 1])
        convb.append(t)
    asc = wpool.tile([128, NTILE], F32, tag="asc", name="asc")
    nc.sync.dma_start(asc[:], d["ascT"][128 * l:128 * (l + 1), :])

    # -------- LN --------
    xn_tiles = _layernorm(tc, r_tiles, lnw, lnb, c272, tmp_pool, psA, big_pool,
                          eps_t)
    if l == 0:
        _probe(tc, "xn0", xn_tiles[0][:])

    # -------- in_proj: xn -> xm (6 tiles, shard-first order) + silu(z) --------
    xm_tiles = []
    sz_tiles = []
    for mi in range(9):
        m0 = 272 * (mi // 3) + MT[mi % 3][0]
        ml = MT[mi % 3][1]
        dest_xm = mi < 6
        if dest_xm:
            ot = big_pool.tile([ml, T + XPAD], F32R, tag=f"xm{mi}",
                               name=f"xm{mi}")
            xm_tiles.append(ot)
            nc.scalar.copy(ot[:, 0:XPAD], zero4[:ml, :])
        else:
            ot = big_pool.tile([ml, T], BF16, tag=f"sz{mi}", name=f"sz{mi}")
            sz_tiles.append(ot)
        for (c0, cl) in _chunks():
            ps = psA.tile([128, 512], F32, tag="mm", name="ip_ps")
            for kc in range(3):
                _mm(nc, ps[:ml, :], win_sb[kc][:, m0:m0 + ml],
                                 xn_tiles[kc][:, c0:c0 + cl],
                                 start=(kc == 0), stop=(kc == 2))
            if dest_xm:
                nc.scalar.copy(ot[:, XPAD + c0:XPAD + c0 + cl], ps[:ml, :])
            else:
                sg = tmp_pool.tile([ml, 512], F32, tag="sg", name="sg")
                nc.scalar.activation(sg[:], ps[:ml, :], AF.Sigmoid)
                nc.vector.tensor_tensor(ot[:, c0:c0 + cl], ps[:ml, :], sg[:],
                                        OP.mult)

    # -------- depthwise causal conv (PE diag matmuls) + silu -> u --------
    u_tiles = []
    for mi, (m0, ml) in enumerate(MT6):
        ut = big_pool.tile([ml, T], F32R, tag=f"u{mi}", name=f"u{mi}")
        u_tiles.append(ut)
        cdg = tmp_pool.tile([ml, 4 * ml], F32R, tag="cdg", name=f"cdg{mi}")
        nc.sync.dma_start(cdg[:],
                          d["convdiag"][544 * l + m0:544 * l + m0 + ml,
                                        0:4 * ml])
        for (c0, cl) in _chunks():
            acc_ps = psY.tile([128, 512], F32, tag="y", name="cv_ps")
            # xm is left-padded with XPAD zero cols: xm_pad[:, XPAD+j] = xm[j],
            # so tap t reads xm_pad[:, c0+1+t : c0+1+t+cl] (causal conv)
            for tap in range(4):
                _mm(nc, acc_ps[:ml, :], cdg[:, tap * ml:tap * ml + ml],
                    xm_tiles[mi][:, c0 + 1 + tap:c0 + 1 + tap + cl],
                    start=(tap == 0), stop=(tap == 3))
            sg = tmp_pool.tile([ml, 512], F32, tag="sg", name="sg")
            nc.scalar.activation(sg[:], acc_ps[:ml, :], AF.Sigmoid,
                                 bias=convb[mi][:])
            nc.vector.scalar_tensor_tensor(ut[:, c0:c0 + cl],
                                           acc_ps[:ml, :],
                                           convb[mi][:], sg[:],
                                           op0=OP.add, op1=OP.mult)

    if l == 0:
        _probe(tc, "xm0", xm_tiles[0][:])
        _probe(tc, "u0", u_tiles[0][:])
        _probe(tc, "sz0", sz_tiles[0][:])

    # -------- xproj: u -> dt rows, B_rep, C_rep --------
    dt_sb = tmp_pool.tile([DTP, T], F32R, tag="cvacc", name="dt_sb", bufs=1)
    brep = big_pool.tile([128, T], BF16, tag="brep", name="brep")
    crep = big_pool.tile([128, T], BF16, tag="crep", name="crep")
    for nm, ot, w in (("dt", dt_sb, DTP), ("B", brep, 128), ("C", crep, 128)):
        for (c0, cl) in _chunks():
            ps = psA.tile([128, 512], F32, tag="mm", name="xp_ps")
            for kc in range(6):
                kl = MT6[kc][1]
                _mm(nc, ps[:w, :], wxp[nm][kc][:, 0:w],
                                 u_tiles[kc][:, c0:c0 + cl],
                                 start=(kc == 0), stop=(kc == 5))
            nc.scalar.copy(ot[:, c0:c0 + cl], ps[:w, :])

    # -------- dt_proj + softplus -> delta; du = delta * u_own --------
    delta_tiles = []
    deltar_tiles = []
    du_tiles = []
    # softplus(x) = ln(1 + exp(x)): batch all EXPs then all LNs; the act
    # table chooser picks first-fit sets, so alternation would reload tables
    for mi, (m0, ml) in enumerate(MT):
        dl_t = big_pool.tile([ml, T + XPAD], F32R, tag=f"xm{mi}",
                             name=f"delta{mi}")
        delta_tiles.append(dl_t)
        deltar_tiles.append(dl_t)
        for (c0, cl) in _chunks():
            ps = psA.tile([128, 512], F32, tag="mm", name="dt_ps")
            _mm(nc, ps[:ml, :], wdt_sb[:, m0:m0 + ml],
                             dt_sb[:, c0:c0 + cl], start=True, stop=True)
            nc.scalar.activation(dl_t[:, c0:c0 + cl], ps[:ml, :], AF.Exp,
                                 bias=vecs["dtb"][mi][:])
    for mi, (m0, ml) in enumerate(MT):
        dl_t = delta_tiles[mi]
        for (c0, cl) in _chunks():
            nc.scalar.activation(dl_t[:, c0:c0 + cl], dl_t[:, c0:c0 + cl],
                                 AF.Ln, bias=1.0)
        du_t = big_pool.tile([ml, T + XPAD], F32R, tag=f"xm{mi + 3}",
                             name=f"du{mi}")
        nc.gpsimd.tensor_tensor(du_t[:, 0:T], dl_t[:, 0:T],
                                u_tiles[mi][:, 0:T], OP.mult)
        du_tiles.append(du_t)

    if l == 0:
        _probe(tc, "dtsb", dt_sb[:])
        _probe(tc, "brep", brep[:])
        _probe(tc, "crep", crep[:])
        _probe(tc, "delta0", delta_tiles[0][:])
        _probe(tc, "du0", du_tiles[0][:])

    # -------- selective scan over 34 (d,n)-tiles --------
    ysz_tiles = {}
    for g in range(3):
        gm0, gml = MT[g]
        y_ps_c = [psY.tile([128, 512], F32, tag="y", name=f"y_ps{c}")
                  for c in range(NC2)]
        k_lo, k_hi = 16 * g, min(16 * g + 16, NTILE)
        # process scan tiles in PAIRS: one [128, 2T] scan + one Pool hC per
        # pair (the a-column at the pair seam is zeroed so the recurrence
        # restarts), halving per-tile instruction and semaphore counts
        for kp in range(k_lo, k_hi, 2):
            rt = kp // 16
            kl = MT[rt][1]
            a_t = scan_pool.tile([128, 2 * T], F32, tag="a", name="a_t",
                                 bufs=2)
            b_t = scan_pool.tile([128, 2 * T], F32, tag="b", name="b_t",
                                 bufs=1)
            h_t = scan_pool.tile([128, 2 * T], F32R, tag="h", name="h_t",
                                 bufs=2)
            for ki in range(2):
                k = kp + ki
                j = k - k_lo
                o0 = ki * T
                dr_ps = []
                for (c0, cl) in _chunks():
                    ps1 = psB.tile([128, 512], F32, tag="rep", name="dr_ps")
                    _mm(nc, ps1[:], repsel[:kl, 128 * j:128 * j + 128],
                                     deltar_tiles[rt][:, c0:c0 + cl],
                                     start=True, stop=True)
                    dr_ps.append(ps1)
                for ci, (c0, cl) in enumerate(_chunks()):
                    nc.scalar.activation(a_t[:, o0 + c0:o0 + c0 + cl],
                                         dr_ps[ci][:], AF.Exp,
                                         scale=asc[:, k:k + 1])
                    ps2 = psB.tile([128, 512], F32, tag="rep", name="du_ps")
                    _mm(nc, ps2[:], repsel[:kl, 128 * j:128 * j + 128],
                                     du_tiles[rt][:, c0:c0 + cl],
                                     start=True, stop=True)
                    nc.vector.tensor_tensor(b_t[:, o0 + c0:o0 + c0 + cl],
                                            ps2[:], brep[:, c0:c0 + cl],
                                            OP.mult)
            nc.vector.memset(a_t[:, T:T + 1], 0.0)
            nc.vector.tensor_tensor_scan(h_t[:], a_t[:], b_t[:], 0.0,
                                         op0=OP.mult, op1=OP.add)
            nc.gpsimd.tensor_tensor(h_t[:, 0:T], h_t[:, 0:T], crep[:],
                                    OP.mult)
            nc.gpsimd.tensor_tensor(h_t[:, T:2 * T], h_t[:, T:2 * T],
                                    crep[:], OP.mult)
            for ki in range(2):
                k = kp + ki
                j = k - k_lo
                o0 = ki * T
                for ci, (c0, cl) in enumerate(_chunks()):
                    _mm(nc, y_ps_c[ci][:gml, :],
                                     yredsel[:, 128 * j:128 * j + gml],
                                     h_t[:, o0 + c0:o0 + c0 + cl],
                                     start=(j == 0), stop=(k == k_hi - 1))
        t2 = big_pool.tile([gml, T], F32R, tag=f"u{3 + g}", name=f"yt{g}")
        for ci, (c0, cl) in enumerate(_chunks()):
            nc.vector.scalar_tensor_tensor(
                t2[:, c0:c0 + cl], u_tiles[g][:, c0:c0 + cl],
                vecs["dp"][g][:], y_ps_c[ci][:gml, :],
                op0=OP.mult, op1=OP.add)
        # on DVE: keeps Pool's FIFO clear ahead of the out_proj AR trigger
        nc.vector.tensor_tensor(t2[:], t2[:], sz_tiles[g][:], OP.mult)
        if l == 0 and g == 0:
            _probe(tc, "ysz0", t2[:])
        ysz_tiles[g] = t2

    # -------- out_proj -> chunked AllReduce -> residual add --------
    # Per 512-col chunk: out_proj, AllReduce, residual add.  Chunk c0's
    # collective overlaps chunk c1's out_proj, and the next layer's LN /
    # in_proj on c0 can start while c1's collective is in flight.
    for ci, (c0, cl) in enumerate(_chunks()):
        ar_in = dram.tile([272, 512], BF16, tag=f"ar_in{l}_{ci}",
                          name=f"ar_in{l}_{ci}")
        ar_out = dram.tile([272, 512], BF16, tag=f"ar_out{l}_{ci}",
                           name=f"ar_out{l}_{ci}")
        for mi, (m0, ml) in enumerate(MT):
            ps = psA.tile([128, 512], F32, tag="mm", name="op_ps")
            for i, kc in enumerate(range(3)):
                _mm(nc, ps[:ml, :], wout_sb[kc][:, m0:m0 + ml],
                                 ysz_tiles[kc][:, c0:c0 + cl],
                                 start=(i == 0), stop=(i == 2))
            op_sb = tmp_pool.tile([ml, 512], BF16, tag="opc", name=f"op{mi}")
            nc.scalar.copy(op_sb[:], ps[:ml, :])
            nc.sync.dma_start(ar_in[m0:m0 + ml, :], op_sb[:])
        nc.gpsimd.collective_compute(
            "AllReduce", OP.add, replica_groups=groups,
            ins=[ar_in.opt()], outs=[ar_out.opt()])
        for mi, (m0, ml) in enumerate(MT):
            os_t = tmp_pool.tile([ml, 512], BF16, tag="osc", name=f"os{mi}")
            nc.sync.dma_start(os_t[:], ar_out[m0:m0 + ml, :])
            # on DVE, not Pool: the next chunk's collective trigger shares
            # Pool's FIFO and must not queue behind these adds
            nc.vector.tensor_tensor(r_tiles[mi][:, c0:c0 + cl],
                                    r_tiles[mi][:, c0:c0 + cl], os_t[:],
                                    OP.add)
    if l == 0:
        _probe(tc, "r0after", r_tiles[0][:])


# ======================= host side =======================

def prep_core_inputs(inputs, core):
    f32 = np.float32
    beta, s = core // 2, core % 2
    g = lambda k: np.asarray(inputs[k], f32)
    type_seq = np.asarray(inputs["type_seq"]).astype(np.int64)
    rows = np.r_[np.arange(272 * s, 272 * s + 272),
                 np.arange(272 * (1 - s), 272 * (1 - s) + 272)]  # own-first xm order
    sh = np.arange(272 * s, 272 * s + 272)

    m = {}
    # embedding gather is pure indexing; tanh stays on device
    m["embrows"] = np.ascontiguousarray(g("emb")[type_seq[beta]].T)
    m["time_row"] = np.ascontiguousarray(g("time_seq")[beta][None, :T])
    div = np.exp(np.arange(0, D_TIME, 2, dtype=f32) * (-(math.log(10000.0) / D_TIME)))
    m["divpat"] = np.tile(div, 2)[None, :].astype(f32)
    m["shiftv"] = np.r_[np.full(8, 0.0), np.full(8, 0.5 * math.pi)][:, None].astype(f32)

    repsel = np.zeros((128, 2048), f32)
    yredsel = np.zeros((128, 2048), f32)
    for j in range(16):
        for p in range(128):
            repsel[8 * j + p // 16, 128 * j + p] = 1.0
            yredsel[p, 128 * j + 8 * j + p // 16] = 1.0
    m["repsel"] = repsel
    m["yredsel"] = yredsel
    c272 = np.empty((272, 2), f32)
    c272[:, 0] = -1.0 / D
    c272[:, 1] = 1.0 / D
    m["c272"] = c272

    A = -np.exp(g("A_log"))  # (4, 544, 16)
    ascT = np.empty((4 * 128, NTILE), f32)
    for l in range(4):
        for k in range(NTILE):
            for p in range(128):
                ascT[128 * l + p, k] = A[l, sh[8 * k + p // 16], p % 16]
    m["ascT"] = ascT

    in_w = g("in_w").copy()       # (4, 1088, 272)
    in_w[:, :, 256:272] = in_w[:, :, 256 + PERM]
    win = np.empty((4 * 272, 816), f32)
    for l in range(4):
        W = in_w[l][np.r_[rows, 544 + sh]]  # (816, 272)
        win[272 * l:272 * (l + 1)] = W.T
    m["win"] = win

    xp = g("xproj_w")             # (4, 49, 544)
    DTP = 18
    wxpdt = np.zeros((4 * 544, DTP), f32)
    wxpB = np.zeros((4 * 544, 128), f32)
    wxpC = np.zeros((4 * 544, 128), f32)
    for l in range(4):
        Wx = xp[l][:, rows]       # (49, 544) reordered columns
        wxpdt[544 * l:544 * (l + 1), :DT_RANK] = Wx[:DT_RANK].T
        for p in range(128):
            wxpB[544 * l:544 * (l + 1), p] = Wx[DT_RANK + p % 16]
            wxpC[544 * l:544 * (l + 1), p] = Wx[DT_RANK + 16 + p % 16]
    m["wxpdt"], m["wxpB"], m["wxpC"] = wxpdt, wxpB, wxpC

    dt_w = g("dt_w")              # (4, 544, 17)
    wdt = np.zeros((4 * DTP, 272), f32)
    for l in range(4):
        wdt[DTP * l:DTP * l + DT_RANK] = dt_w[l][sh].T
    m["wdt"] = wdt

    out_w = g("out_w").copy()     # (4, 272, 544)
    out_w[:, 256:272, :] = out_w[:, 256 + PERM, :]
    wout = np.empty((4 * 272, 272), f32)
    for l in range(4):
        wout[272 * l:272 * (l + 1)] = out_w[l][:, sh].T
    m["wout"] = wout

    cw = g("conv_w")[:, :, 0, :]  # (4, 544, 4)
    convdiag = np.zeros((4 * 544, 512), f32)
    mt6 = [(0, 128), (128, 128), (256, 16), (272, 128), (400, 128), (528, 16)]
    for l in range(4):
        base = cw[l][rows]  # (544, 4) own-first channel order
        for (m0, ml) in mt6:
            for tap in range(4):
                blk = convdiag[544 * l + m0:544 * l + m0 + ml,
                               tap * ml:(tap + 1) * ml]
                np.fill_diagonal(blk, base[m0:m0 + ml, tap])
    m["convdiag"] = convdiag

    m["dtb"] = np.ascontiguousarray(g("dt_b")[:, sh].T)          # (272, 4)
    m["convb"] = np.ascontiguousarray(g("conv_b")[:, rows].T)    # (544, 4)
    m["dparam"] = np.ascontiguousarray(g("D_param")[:, sh].T)    # (272, 4)

    lnw = g("norm_w").copy()
    lnb = g("norm_b").copy()
    lnw[:, 256:272] = lnw[:, 256 + PERM]
    lnb[:, 256:272] = lnb[:, 256 + PERM]
    m["lnw"] = lnw
    m["lnbT"] = np.ascontiguousarray(lnb.T)
    lnfw = g("normf_w").copy()
    lnfb = g("normf_b").copy()
    lnfw[256:272] = lnfw[256 + PERM]
    lnfb[256:272] = lnfb[256 + PERM]
    m["lnfw"] = lnfw[None, :]
    m["lnfbT"] = lnfb[:, None]
    return m


def assemble_output(per_core_results):
    """per_core_results: list of 8 dicts with 'out_fm' (272, 1024)."""
    outs = []
    for b in range(B):
        o = np.asarray(per_core_results[2 * b]["out_fm"], np.float32)
        oo = o.copy()
        oo[256 + PERM] = o[256:272]
        outs.append(oo.T)
    return np.stack(outs).astype(np.float32)


_PROG = {}


def _get_program(n_cores=8):
    if n_cores not in _PROG:
        _PROG[n_cores] = build_program(n_cores)
    return _PROG[n_cores]


def run(inputs, trace=False, trace_kwargs=None):
    nc = _get_program(8)
    in_maps = [prep_core_inputs(inputs, c) for c in range(8)]
    from concourse.bass_utils import run_bass_kernel_spmd
    res = run_bass_kernel_spmd(nc, in_maps, core_ids=list(range(8)),
                               trace=trace, **(trace_kwargs or {}))
    return assemble_output(res.results), res


def kernel(**inputs):
    out, _ = run(inputs)
    return out

